# revision 1
# baseline (speedup 1.0000x reference)
"""GNN message-passing (NodeModel) Trainium2 kernel, 8 NeuronCores.

Sharding: edges partitioned by destination node (12500 nodes/core) -> the
segment-sum stays core-local, no collectives. Per core, edges are grouped by
source half-bank (8 x 12544 nodes), sorted by destination, and laid out in
fixed 384-slot cells per (half-bank, 128-dest-block) so every core runs the
identical program (SPMD, one NEFF).

Math (W1b deferred out of the edge loop):
  z_e   = P[col_e] + ea_e @ B + b1a,   P = x @ W1a[:32]
  m_e   = relu(z_e);  S_n = sum_e m_e;  c_n = deg(n)
  agg_n = (S_n / max(c_n,1)) @ W1b + 1[c_n>0] * b1b
  out   = relu([x | agg] @ W2a + b2a) @ W2b + b2b

The per-edge P fetch avoids DMA gather entirely (random 256B HBM gathers
measured ~31 GB/s, latency-bound): P^T for one half-bank lives in SBUF
feature-major [64, 12544] f32 (computed on device), and gpsimd ap_gather
expands it per-slot along the free dim. Partitions 64:128 hold a copy of the
table so the 8 Q7 cores serve two slot-halves per gathered column (column c
-> rows 0:64 = slot c, rows 64:128 = slot c + CHUNK/2). z is accumulated in
PSUM from two matmuls (gathered-P against I64, edge attrs against Btil); S
and c are accumulated with one-hot matmuls into PSUM; b1b is folded into
W2a_eff with the count indicator as a 97th feature.
"""
import numpy as np
import ml_dtypes

N_NODES = 100000
F = 32
HID = 64
NTGT = 32
NCORES = 8

NSH = 12500            # dest nodes per core
NSH_PAD = 12544        # 98 * 128
NBLK = 98              # dest-node blocks (128 nodes) per core
HB = 8                 # source half-banks
HB_N = 12544           # source nodes per half-bank
CELL = 384             # slots per (half-bank, dest-block) = 3 tiles of 128
TPB = 3
NCELL_TOT = HB * NBLK              # 784
S_TOT = NCELL_TOT * CELL           # 301056
CHUNK_CELLS = 7
CHUNK = CHUNK_CELLS * CELL         # 2688 slots (= one gather call)
CHUNKS_PER_HB = NBLK // CHUNK_CELLS    # 14
HB_SLOTS = NBLK * CELL                 # 37632
OFFS_NONE = 200.0
EAR = F + 1            # ea rows (32 feats + indicator)

_CACHE = {}


def _build_nc():
    import os
    import concourse.bass as bass
    import concourse.bacc as bacc
    import concourse.mybir as mybir
    from concourse.tile import TileContext
    from bass_rust import add_dep_helper

    f32 = mybir.dt.float32
    bf16 = mybir.dt.bfloat16
    i16 = mybir.dt.int16
    AF = mybir.ActivationFunctionType
    OP = mybir.AluOpType

    nc = bacc.Bacc("TRN2", target_bir_lowering=False, debug=False,
                   num_devices=NCORES)

    xT = nc.dram_tensor("xT", [F, HB * HB_N], bf16, kind="ExternalInput")
    x_ownT = nc.dram_tensor("x_ownT", [F, NSH_PAD], f32, kind="ExternalInput")
    A_w = nc.dram_tensor("A_w", [F, 128], bf16, kind="ExternalInput")
    I64_w = nc.dram_tensor("I64_w", [128, HID + 1], f32, kind="ExternalInput")
    Btil_w = nc.dram_tensor("Btil_w", [EAR, HID + 1], bf16,
                            kind="ExternalInput")
    W1b_w = nc.dram_tensor("W1b_w", [HID, HID], f32, kind="ExternalInput")
    W2a_w = nc.dram_tensor("W2a_w", [F + HID + 1, HID], f32, kind="ExternalInput")
    b2a_w = nc.dram_tensor("b2a_w", [HID, 1], f32, kind="ExternalInput")
    W2b_w = nc.dram_tensor("W2b_w", [HID, NTGT], bf16, kind="ExternalInput")
    b2b_w = nc.dram_tensor("b2b_w", [NTGT, 1], f32, kind="ExternalInput")
    iota_w = nc.dram_tensor("iota_w", [128, 128], bf16, kind="ExternalInput")
    idf_w = nc.dram_tensor("idf_w", [128, 128], f32, kind="ExternalInput")
    gidx_w = nc.dram_tensor("gidx_w", [128, S_TOT // 16], i16,
                            kind="ExternalInput")
    eaT_w = nc.dram_tensor("eaT_w", [EAR, S_TOT], bf16, kind="ExternalInput")
    offs_w = nc.dram_tensor("offs_w", [128, S_TOT // 128], bf16,
                            kind="ExternalInput")
    outT = nc.dram_tensor("outT", [NTGT, NSH_PAD], f32, kind="ExternalOutput")

    with TileContext(nc) as tc:
        with tc.tile_pool(name="const", bufs=1) as cpool, \
             tc.tile_pool(name="acc", bufs=1) as apool, \
             tc.tile_pool(name="tbl", bufs=1) as tpool:

            # constants
            A_sb = cpool.tile([F, 128], bf16)
            nc.sync.dma_start(out=A_sb[:], in_=A_w[:])
            I64_sb = cpool.tile([128, HID + 1], f32)
            nc.sync.dma_start(out=I64_sb[:], in_=I64_w[:])
            Btil_sb = cpool.tile([EAR, HID + 1], bf16)
            nc.sync.dma_start(out=Btil_sb[:], in_=Btil_w[:])
            W1b_sb = cpool.tile([HID, HID], f32)
            nc.sync.dma_start(out=W1b_sb[:], in_=W1b_w[:])
            W2a_sb = cpool.tile([F + HID + 1, HID], f32)
            nc.sync.dma_start(out=W2a_sb[:], in_=W2a_w[:])
            b2a_sb = cpool.tile([HID, 1], f32)
            nc.sync.dma_start(out=b2a_sb[:], in_=b2a_w[:])
            W2b_sb = cpool.tile([HID, NTGT], bf16)
            nc.sync.dma_start(out=W2b_sb[:], in_=W2b_w[:])
            b2b_sb = cpool.tile([NTGT, 1], f32)
            nc.sync.dma_start(out=b2b_sb[:], in_=b2b_w[:])
            iota_sb = cpool.tile([128, 128], bf16)
            nc.sync.dma_start(out=iota_sb[:], in_=iota_w[:])
            offs_sb = cpool.tile([128, S_TOT // 128], bf16)
            nc.sync.dma_start(out=offs_sb[:], in_=offs_w[:])
            idf = cpool.tile([128, 128], f32)
            nc.sync.dma_start(out=idf[:], in_=idf_w[:])

            acc_sb = apool.tile([128, NSH_PAD], f32)
            nc.vector.memset(acc_sb[:], 0.0)

            tbl = tpool.tile([128, HB_N, 1], f32)

            with tc.tile_pool(name="xtp", bufs=2) as xpool, \
                 tc.tile_pool(name="pbp", bufs=2, space="PSUM") as pbppool, \
                 tc.tile_pool(name="gidx", bufs=2) as gxpool, \
                 tc.tile_pool(name="gt", bufs=3) as gpool, \
                 tc.tile_pool(name="ea", bufs=2) as eapool, \
                 tc.tile_pool(name="msg", bufs=3) as mpool, \
                 tc.tile_pool(name="oh", bufs=3) as ohpool, \
                 tc.tile_pool(name="ebp", bufs=4, space="PSUM") as ebppool, \
                 tc.tile_pool(name="accp", bufs=2, space="PSUM") as accppool:

                for b in range(HB):
                    # ---- table build: P^T = A^T @ x^T, feature-major ----
                    for g0 in range(0, HB_N, 6272):
                        xT_sb = xpool.tile([F, 6272], bf16, tag="xT")
                        nc.sync.dma_start(
                            out=xT_sb[:],
                            in_=xT[:, b * HB_N + g0:b * HB_N + g0 + 6272])
                        for h0 in range(0, 6272, 512):
                            n = min(512, 6272 - h0)
                            ps = pbppool.tile([128, 512], f32, tag="pbp")
                            for q0 in range(0, n, 128):
                                nc.tensor.matmul(
                                    ps[:, q0:q0 + 128], A_sb[:],
                                    xT_sb[:, h0 + q0:h0 + q0 + 128],
                                    start=True, stop=True)
                            nc.scalar.activation(
                                tbl[:, g0 + h0:g0 + h0 + n, 0],
                                ps[:, 0:n], AF.Copy)

                    gidx_sb = gxpool.tile([128, HB_SLOTS // 16], i16,
                                          tag="gidx")
                    nc.sync.dma_start(
                        out=gidx_sb[:],
                        in_=gidx_w[:, b * (HB_SLOTS // 16):
                                   (b + 1) * (HB_SLOTS // 16)])

                    if os.environ.get("SKIP_P2"):
                        continue
                    for c in range(CHUNKS_PER_HB):
                        gt = gpool.tile([128, CHUNK, 1], f32, tag="gt")
                        if os.environ.get("APG_OFF"):
                            nc.vector.memset(gt[:], 0.0)
                        else:
                            gth = nc.gpsimd.ap_gather(
                                gt[:], tbl[:],
                                gidx_sb[:, c * (CHUNK // 16):
                                        (c + 1) * (CHUNK // 16)],
                                128, HB_N, 1, CHUNK)

                        slot0 = b * HB_SLOTS + c * CHUNK
                        ea_t = eapool.tile([EAR, CHUNK], bf16, tag="ea")
                        nc.sync.dma_start(out=ea_t[:],
                                          in_=eaT_w[:, slot0:slot0 + CHUNK])

                        for ci in range(CHUNK_CELLS):
                            dblk = c * CHUNK_CELLS + ci
                            zb = ebppool.tile([128, TPB * (HID + 1)], f32,
                                              tag="ebp")
                            for t in range(TPB):
                                s = ci * CELL + t * 128       # chunk slot
                                nc.tensor.matmul(
                                    zb[:, t * (HID + 1):(t + 1) * (HID + 1)],
                                    gt[0:HID, s:s + 128, 0],
                                    I64_sb[0:HID, :],
                                    start=True, stop=False)
                                nc.tensor.matmul(
                                    zb[:, t * (HID + 1):(t + 1) * (HID + 1)],
                                    ea_t[:, s:s + 128],
                                    Btil_sb[:], start=False, stop=True)
                            zv = zb[:].rearrange("p (t e) -> p t e", e=HID + 1)
                            msg = mpool.tile([128, TPB, HID + 1], bf16,
                                             tag="msg")
                            nc.scalar.activation(msg[:], zv, AF.Relu)

                            oh = ohpool.tile([128, TPB, 128], bf16, tag="oh")
                            tcol = (b * NBLK + dblk) * TPB
                            o_ap = offs_sb[:, tcol:tcol + TPB]
                            o_b = bass.AP(o_ap.tensor, o_ap.offset,
                                          [o_ap.ap[0], o_ap.ap[1], [0, 128]])
                            i_ap = iota_sb[:]
                            i_b = bass.AP(i_ap.tensor, i_ap.offset,
                                          [i_ap.ap[0], [0, TPB], i_ap.ap[1]])
                            nc.vector.tensor_tensor(out=oh[:], in0=o_b,
                                                    in1=i_b, op=OP.is_equal)

                            accp = accppool.tile([HID + 1, 128], f32,
                                                 tag="accp")
                            for t in range(TPB):
                                nc.tensor.matmul(accp[:], msg[:, t, :],
                                                 oh[:, t, :],
                                                 start=(t == 0),
                                                 stop=(t == TPB - 1))
                            nc.vector.tensor_tensor(
                                out=acc_sb[0:HID + 1,
                                           dblk * 128:(dblk + 1) * 128],
                                in0=acc_sb[0:HID + 1,
                                           dblk * 128:(dblk + 1) * 128],
                                in1=accp[:], op=OP.add)

            # ---- Phase 3: node MLP (streamed, 256 nodes per group) ----
            with tc.tile_pool(name="node", bufs=3) as npool, \
                 tc.tile_pool(name="nodep", bufs=1, space="PSUM") as nppool:
                for G in range(NSH_PAD // 256):         # 49 groups
                    # rows: 0:64 Agg^T, 64 ind^T, 65:97 x_own^T
                    rhs2 = npool.tile([F + HID + 1, 256], f32, tag="rhs2")
                    nc.sync.dma_start(out=rhs2[HID + 1:HID + 1 + F, :],
                                      in_=x_ownT[:, G * 256:(G + 1) * 256])
                    for j in range(2):
                        q = 2 * G + j
                        pS = nppool.tile([128, 128], f32, tag="pS")
                        nc.tensor.transpose(
                            out=pS[:], in_=acc_sb[:, q * 128:(q + 1) * 128],
                            identity=idf[:])
                        rec = npool.tile([128, 1], f32, tag="rec")
                        nc.vector.tensor_scalar_max(rec[:], pS[:, HID:HID + 1],
                                                    1.0)
                        nc.vector.reciprocal(rec[:], rec[:])
                        ind = npool.tile([128, 1], f32, tag="ind")
                        nc.vector.tensor_scalar_min(ind[:], pS[:, HID:HID + 1],
                                                    1.0)
                        pA = nppool.tile([128, HID], f32, tag="pA")
                        nc.tensor.matmul(pA[:],
                                         acc_sb[0:HID, q * 128:(q + 1) * 128],
                                         W1b_sb[:], start=True, stop=True)
                        tq = npool.tile([128, 128], f32, tag="tq")
                        nc.vector.tensor_scalar_mul(tq[:, 0:HID], pA[:], rec[:])
                        nc.vector.tensor_copy(out=tq[:, HID:HID + 1], in_=ind[:])
                        nc.vector.memset(tq[:, HID + 1:128], 0.0)
                        pT = nppool.tile([128, 128], f32, tag="pT")
                        nc.tensor.transpose(out=pT[:], in_=tq[:], identity=idf[:])
                        nc.vector.tensor_copy(
                            out=rhs2[0:HID + 1, j * 128:(j + 1) * 128],
                            in_=pT[0:HID + 1, :])
                    ph = nppool.tile([HID, 256], f32, tag="ph")
                    nc.tensor.matmul(ph[:], W2a_sb[:], rhs2[:],
                                     start=True, stop=True)
                    h1 = npool.tile([HID, 256], bf16, tag="h1")
                    nc.scalar.activation(h1[:], ph[:], AF.Relu, bias=b2a_sb[:])
                    po = nppool.tile([NTGT, 256], f32, tag="po")
                    nc.tensor.matmul(po[:], W2b_sb[:], h1[:],
                                     start=True, stop=True)
                    ot = npool.tile([NTGT, 256], f32, tag="ot")
                    nc.scalar.activation(ot[:], po[:], AF.Identity,
                                         bias=b2b_sb[:])
                    nc.sync.dma_start(out=outT[:, G * 256:(G + 1) * 256],
                                      in_=ot[:])

    nc.compile()
    return nc


def _host_prep(x, edge_index, edge_attr, W1a, b1a, W1b, b1b, W2a, b2a,
               W2b, b2b):
    bf = ml_dtypes.bfloat16
    row = np.asarray(edge_index[0], dtype=np.int64)
    col = np.asarray(edge_index[1], dtype=np.int64)
    x = np.asarray(x, dtype=np.float32)
    ea = np.asarray(edge_attr, dtype=np.float32)

    xT_pad = np.zeros((F, HB * HB_N), bf)
    xT_pad[:, :N_NODES] = x.T

    I64e = np.zeros((128, HID + 1), np.float32)
    I64e[0:HID, 0:HID] = np.eye(HID)
    I64e[HID:128, 0:HID] = np.eye(HID)

    Btil = np.zeros((EAR, HID + 1), np.float32)
    Btil[0:F, 0:HID] = W1a[F:, :]
    Btil[F, 0:HID] = b1a
    Btil[F, HID] = 1.0

    # rhs2 row order: [Agg (64); ind (1); x (32)]
    W2a_eff = np.concatenate(
        [W2a[F:, :], (b1b @ W2a[F:, :]).reshape(1, HID), W2a[:F, :]],
        axis=0).astype(np.float32)

    iota = np.tile(np.arange(128, dtype=np.float32), (128, 1)).astype(bf)

    common = {
        "xT": xT_pad,
        "A_w": np.tile(np.asarray(W1a[:F, :], np.float32), (1, 2)).astype(bf),
        "I64_w": I64e,
        "Btil_w": Btil.astype(bf),
        "W1b_w": np.asarray(W1b, np.float32),
        "W2a_w": W2a_eff,
        "b2a_w": np.asarray(b2a, np.float32).reshape(HID, 1),
        "b2b_w": np.asarray(b2b, np.float32).reshape(NTGT, 1),
        "W2b_w": np.asarray(W2b, np.float32).astype(bf),
        "iota_w": iota,
        "idf_w": np.eye(128, dtype=np.float32),
    }

    shard = row // NSH
    in_maps = []
    for core in range(NCORES):
        sel = np.nonzero(shard == core)[0]
        lrow = (row[sel] - core * NSH).astype(np.int64)
        scol = col[sel]
        hb = scol // HB_N
        lcol = (scol - hb * HB_N).astype(np.int64)
        blk = lrow >> 7
        bid = hb * NBLK + blk
        order = np.lexsort((lrow, bid))
        sbid = bid[order]
        cnt = np.bincount(bid, minlength=NCELL_TOT)
        mx = cnt.max()
        assert mx <= CELL, f"cell overflow: {mx} > {CELL}"
        starts = np.zeros(NCELL_TOT + 1, np.int64)
        starts[1:] = np.cumsum(cnt)
        within = np.arange(len(order)) - starts[sbid]
        slots = sbid * CELL + within

        gidx = np.zeros(S_TOT, np.int16)          # poison -> node 0
        gidx[slots] = lcol[order].astype(np.int16)
        eaT = np.zeros((EAR, S_TOT), bf)
        eaT[0:F, slots] = ea[sel][order].T
        eaT[F, slots] = 1.0
        offs = np.full(S_TOT, OFFS_NONE, np.float32)
        offs[slots] = (lrow[order] & 127).astype(np.float32)

        # gather idx columns: col = slot, wrapped in 16 partitions and
        # replicated to all 8 q7 core groups
        gw = np.tile(gidx.reshape(S_TOT // 16, 16).T, (8, 1))

        x_ownT = np.zeros((F, NSH_PAD), np.float32)
        x_ownT[:, :NSH] = x[core * NSH:(core + 1) * NSH].T

        m = dict(common)
        m["x_ownT"] = x_ownT
        m["gidx_w"] = gw
        m["eaT_w"] = eaT
        m["offs_w"] = offs.reshape(S_TOT // 128, 128).T.astype(bf).copy()
        in_maps.append(m)
    return in_maps


def kernel(x, edge_index, edge_attr, u, batch,
           W1a, b1a, W1b, b1b, W2a, b2a, W2b, b2b, _profile=False):
    from concourse import bass_utils

    if "nc" not in _CACHE:
        _CACHE["nc"] = _build_nc()
    nc = _CACHE["nc"]

    in_maps = _host_prep(x, edge_index, edge_attr, W1a, b1a, W1b, b1b,
                         W2a, b2a, W2b, b2b)
    res = bass_utils.run_bass_kernel_spmd(
        nc, in_maps, core_ids=list(range(NCORES)), trace=_profile)
    out = np.empty((N_NODES, NTGT), np.float32)
    for core in range(NCORES):
        out[core * NSH:(core + 1) * NSH] = \
            res.results[core]["outT"][:, :NSH].T
    if _profile:
        _CACHE["last_exec_time_ns"] = res.exec_time_ns
    return out



# revision 2
# speedup vs baseline: 1.1078x; 1.1078x over previous
"""GNN message-passing (NodeModel) Trainium2 kernel, 8 NeuronCores.

Sharding: edges partitioned by destination node (12500 nodes/core) -> the
segment-sum stays core-local, no collectives. Per core, edges are grouped by
source half-bank (8 x 12544 nodes), sorted by destination, and laid out in
fixed 384-slot cells per (half-bank, 128-dest-block) so every core runs the
identical program (SPMD, one NEFF).

Math (W1b deferred out of the edge loop):
  z_e   = P[col_e] + ea_e @ B + b1a,   P = x @ W1a[:32]
  m_e   = relu(z_e);  S_n = sum_e m_e;  c_n = deg(n)
  agg_n = (S_n / max(c_n,1)) @ W1b + 1[c_n>0] * b1b
  out   = relu([x | agg] @ W2a + b2a) @ W2b + b2b

The per-edge P fetch avoids DMA gather entirely (random 256B HBM gathers
measured ~31 GB/s, latency-bound): P^T for one half-bank lives in SBUF
feature-major [64, 12544] f32 (computed on device), and gpsimd ap_gather
expands it per-slot along the free dim. Partitions 64:128 hold a copy of the
table so the 8 Q7 cores serve two slot-halves per gathered column (column c
-> rows 0:64 = slot c, rows 64:128 = slot c + CHUNK/2). z is accumulated in
PSUM from two matmuls (gathered-P against I64, edge attrs against Btil); S
and c are accumulated with one-hot matmuls into PSUM; b1b is folded into
W2a_eff with the count indicator as a 97th feature.
"""
import numpy as np
import ml_dtypes

N_NODES = 100000
F = 32
HID = 64
NTGT = 32
NCORES = 8

NSH = 12500            # dest nodes per core
NSH_PAD = 12544        # 98 * 128
NBLK = 98              # dest-node blocks (128 nodes) per core
HB = 8                 # source half-banks
HB_N = 12544           # source nodes per half-bank
CELL = 384             # slots per (half-bank, dest-block) = 3 tiles of 128
TPB = 3
NCELL_TOT = HB * NBLK              # 784
S_TOT = NCELL_TOT * CELL           # 301056
CHUNK_CELLS = 7
CHUNK = CHUNK_CELLS * CELL         # 2688 slots (= one gather call)
CHUNKS_PER_HB = NBLK // CHUNK_CELLS    # 14
HB_SLOTS = NBLK * CELL                 # 37632
OFFS_NONE = 200.0
EAR = F + 1            # ea rows (32 feats + indicator)

_CACHE = {}


def _build_nc():
    import os
    import concourse.bass as bass
    import concourse.bacc as bacc
    import concourse.mybir as mybir
    from concourse.tile import TileContext
    from bass_rust import add_dep_helper

    f32 = mybir.dt.float32
    bf16 = mybir.dt.bfloat16
    i16 = mybir.dt.int16
    AF = mybir.ActivationFunctionType
    OP = mybir.AluOpType

    nc = bacc.Bacc("TRN2", target_bir_lowering=False, debug=False,
                   num_devices=NCORES)

    xT = nc.dram_tensor("xT", [F, HB * HB_N], bf16, kind="ExternalInput")
    x_ownT = nc.dram_tensor("x_ownT", [F, NSH_PAD], f32, kind="ExternalInput")
    A_w = nc.dram_tensor("A_w", [F, 128], bf16, kind="ExternalInput")
    I64_w = nc.dram_tensor("I64_w", [128, HID + 1], bf16, kind="ExternalInput")
    Btil_w = nc.dram_tensor("Btil_w", [EAR, HID + 1], bf16,
                            kind="ExternalInput")
    W1b_w = nc.dram_tensor("W1b_w", [HID, HID], f32, kind="ExternalInput")
    W2a_w = nc.dram_tensor("W2a_w", [F + HID + 1, HID], f32, kind="ExternalInput")
    b2a_w = nc.dram_tensor("b2a_w", [HID, 1], f32, kind="ExternalInput")
    W2b_w = nc.dram_tensor("W2b_w", [HID, NTGT], bf16, kind="ExternalInput")
    b2b_w = nc.dram_tensor("b2b_w", [NTGT, 1], f32, kind="ExternalInput")
    iota_w = nc.dram_tensor("iota_w", [128, 128], bf16, kind="ExternalInput")
    idf_w = nc.dram_tensor("idf_w", [128, 128], f32, kind="ExternalInput")
    gidx_w = nc.dram_tensor("gidx_w", [128, S_TOT // 16], i16,
                            kind="ExternalInput")
    eaT_w = nc.dram_tensor("eaT_w", [EAR, S_TOT], bf16, kind="ExternalInput")
    offs_w = nc.dram_tensor("offs_w", [128, S_TOT // 128], bf16,
                            kind="ExternalInput")
    outT = nc.dram_tensor("outT", [NTGT, NSH_PAD], f32, kind="ExternalOutput")

    with TileContext(nc) as tc:
        with tc.tile_pool(name="const", bufs=1) as cpool, \
             tc.tile_pool(name="acc", bufs=1) as apool, \
             tc.tile_pool(name="tbl", bufs=1) as tpool:

            # constants
            A_sb = cpool.tile([F, 128], bf16)
            nc.sync.dma_start(out=A_sb[:], in_=A_w[:])
            I64_sb = cpool.tile([128, HID + 1], bf16)
            nc.sync.dma_start(out=I64_sb[:], in_=I64_w[:])
            Btil_sb = cpool.tile([EAR, HID + 1], bf16)
            nc.sync.dma_start(out=Btil_sb[:], in_=Btil_w[:])
            W1b_sb = cpool.tile([HID, HID], f32)
            nc.sync.dma_start(out=W1b_sb[:], in_=W1b_w[:])
            W2a_sb = cpool.tile([F + HID + 1, HID], f32)
            nc.sync.dma_start(out=W2a_sb[:], in_=W2a_w[:])
            b2a_sb = cpool.tile([HID, 1], f32)
            nc.sync.dma_start(out=b2a_sb[:], in_=b2a_w[:])
            W2b_sb = cpool.tile([HID, NTGT], bf16)
            nc.sync.dma_start(out=W2b_sb[:], in_=W2b_w[:])
            b2b_sb = cpool.tile([NTGT, 1], f32)
            nc.sync.dma_start(out=b2b_sb[:], in_=b2b_w[:])
            iota_sb = cpool.tile([128, 128], bf16)
            nc.sync.dma_start(out=iota_sb[:], in_=iota_w[:])
            offs_sb = cpool.tile([128, S_TOT // 128], bf16)
            nc.sync.dma_start(out=offs_sb[:], in_=offs_w[:])
            idf = cpool.tile([128, 128], f32)
            nc.sync.dma_start(out=idf[:], in_=idf_w[:])

            acc_sb = apool.tile([128, NSH_PAD], f32)
            nc.vector.memset(acc_sb[:], 0.0)

            tbl = tpool.tile([128, HB_N, 1], f32)

            with tc.tile_pool(name="xtp", bufs=2) as xpool, \
                 tc.tile_pool(name="pbp", bufs=2, space="PSUM") as pbppool, \
                 tc.tile_pool(name="gidx", bufs=2) as gxpool, \
                 tc.tile_pool(name="gt", bufs=3) as gpool, \
                 tc.tile_pool(name="ea", bufs=2) as eapool, \
                 tc.tile_pool(name="msg", bufs=3) as mpool, \
                 tc.tile_pool(name="oh", bufs=3) as ohpool, \
                 tc.tile_pool(name="ebp", bufs=4, space="PSUM") as ebppool, \
                 tc.tile_pool(name="accp", bufs=2, space="PSUM") as accppool:

                for b in range(HB):
                    # ---- table build: P^T = A^T @ x^T, feature-major ----
                    for g0 in range(0, HB_N, 6272):
                        xT_sb = xpool.tile([F, 6272], bf16, tag="xT")
                        nc.sync.dma_start(
                            out=xT_sb[:],
                            in_=xT[:, b * HB_N + g0:b * HB_N + g0 + 6272])
                        for h0 in range(0, 6272, 512):
                            n = min(512, 6272 - h0)
                            ps = pbppool.tile([128, 512], f32, tag="pbp")
                            for q0 in range(0, n, 128):
                                nc.tensor.matmul(
                                    ps[:, q0:q0 + 128], A_sb[:],
                                    xT_sb[:, h0 + q0:h0 + q0 + 128],
                                    start=True, stop=True)
                            nc.scalar.activation(
                                tbl[:, g0 + h0:g0 + h0 + n, 0],
                                ps[:, 0:n], AF.Copy)

                    gidx_sb = gxpool.tile([128, HB_SLOTS // 16], i16,
                                          tag="gidx")
                    nc.sync.dma_start(
                        out=gidx_sb[:],
                        in_=gidx_w[:, b * (HB_SLOTS // 16):
                                   (b + 1) * (HB_SLOTS // 16)])

                    if os.environ.get("SKIP_P2"):
                        continue
                    for c in range(CHUNKS_PER_HB):
                        gt = gpool.tile([128, CHUNK, 1], f32, tag="gt")
                        if os.environ.get("APG_OFF"):
                            nc.vector.memset(gt[:], 0.0)
                        else:
                            gth = nc.gpsimd.ap_gather(
                                gt[:], tbl[:],
                                gidx_sb[:, c * (CHUNK // 16):
                                        (c + 1) * (CHUNK // 16)],
                                128, HB_N, 1, CHUNK)

                        gtb = gpool.tile([128, CHUNK], bf16, tag="gtb")
                        if c % 2 == 0:
                            nc.vector.tensor_copy(out=gtb[:],
                                                  in_=gt[:, :, 0])
                        else:
                            nc.scalar.activation(gtb[:], gt[:, :, 0],
                                                 AF.Copy)

                        slot0 = b * HB_SLOTS + c * CHUNK
                        ea_t = eapool.tile([EAR, CHUNK], bf16, tag="ea")
                        nc.sync.dma_start(out=ea_t[:],
                                          in_=eaT_w[:, slot0:slot0 + CHUNK])

                        for ci in range(CHUNK_CELLS):
                            dblk = c * CHUNK_CELLS + ci
                            zb = ebppool.tile([128, TPB * (HID + 1)], f32,
                                              tag="ebp")
                            for t in range(TPB):
                                s = ci * CELL + t * 128       # chunk slot
                                nc.tensor.matmul(
                                    zb[:, t * (HID + 1):(t + 1) * (HID + 1)],
                                    gtb[0:HID, s:s + 128],
                                    I64_sb[0:HID, :],
                                    start=True, stop=False)
                                nc.tensor.matmul(
                                    zb[:, t * (HID + 1):(t + 1) * (HID + 1)],
                                    ea_t[:, s:s + 128],
                                    Btil_sb[:], start=False, stop=True)
                            zv = zb[:].rearrange("p (t e) -> p t e", e=HID + 1)
                            msg = mpool.tile([128, TPB, HID + 1], bf16,
                                             tag="msg")
                            nc.scalar.activation(msg[:], zv, AF.Relu)

                            oh = ohpool.tile([128, TPB, 128], bf16, tag="oh")
                            tcol = (b * NBLK + dblk) * TPB
                            o_ap = offs_sb[:, tcol:tcol + TPB]
                            o_b = bass.AP(o_ap.tensor, o_ap.offset,
                                          [o_ap.ap[0], o_ap.ap[1], [0, 128]])
                            i_ap = iota_sb[:]
                            i_b = bass.AP(i_ap.tensor, i_ap.offset,
                                          [i_ap.ap[0], [0, TPB], i_ap.ap[1]])
                            nc.vector.tensor_tensor(out=oh[:], in0=o_b,
                                                    in1=i_b, op=OP.is_equal)

                            accp = accppool.tile([HID + 1, 128], f32,
                                                 tag="accp")
                            for t in range(TPB):
                                nc.tensor.matmul(accp[:], msg[:, t, :],
                                                 oh[:, t, :],
                                                 start=(t == 0),
                                                 stop=(t == TPB - 1))
                            nc.vector.tensor_tensor(
                                out=acc_sb[0:HID + 1,
                                           dblk * 128:(dblk + 1) * 128],
                                in0=acc_sb[0:HID + 1,
                                           dblk * 128:(dblk + 1) * 128],
                                in1=accp[:], op=OP.add)

            # ---- Phase 3: node MLP (streamed, 256 nodes per group) ----
            with tc.tile_pool(name="node", bufs=3) as npool, \
                 tc.tile_pool(name="nodep", bufs=1, space="PSUM") as nppool:
                for G in range(NSH_PAD // 256):         # 49 groups
                    # rows: 0:64 Agg^T, 64 ind^T, 65:97 x_own^T
                    rhs2 = npool.tile([F + HID + 1, 256], f32, tag="rhs2")
                    nc.sync.dma_start(out=rhs2[HID + 1:HID + 1 + F, :],
                                      in_=x_ownT[:, G * 256:(G + 1) * 256])
                    for j in range(2):
                        q = 2 * G + j
                        pS = nppool.tile([128, 128], f32, tag="pS")
                        nc.tensor.transpose(
                            out=pS[:], in_=acc_sb[:, q * 128:(q + 1) * 128],
                            identity=idf[:])
                        rec = npool.tile([128, 1], f32, tag="rec")
                        nc.vector.tensor_scalar_max(rec[:], pS[:, HID:HID + 1],
                                                    1.0)
                        nc.vector.reciprocal(rec[:], rec[:])
                        ind = npool.tile([128, 1], f32, tag="ind")
                        nc.vector.tensor_scalar_min(ind[:], pS[:, HID:HID + 1],
                                                    1.0)
                        pA = nppool.tile([128, HID], f32, tag="pA")
                        nc.tensor.matmul(pA[:],
                                         acc_sb[0:HID, q * 128:(q + 1) * 128],
                                         W1b_sb[:], start=True, stop=True)
                        tq = npool.tile([128, 128], f32, tag="tq")
                        nc.vector.tensor_scalar_mul(tq[:, 0:HID], pA[:], rec[:])
                        nc.vector.tensor_copy(out=tq[:, HID:HID + 1], in_=ind[:])
                        nc.vector.memset(tq[:, HID + 1:128], 0.0)
                        pT = nppool.tile([128, 128], f32, tag="pT")
                        nc.tensor.transpose(out=pT[:], in_=tq[:], identity=idf[:])
                        nc.vector.tensor_copy(
                            out=rhs2[0:HID + 1, j * 128:(j + 1) * 128],
                            in_=pT[0:HID + 1, :])
                    ph = nppool.tile([HID, 256], f32, tag="ph")
                    nc.tensor.matmul(ph[:], W2a_sb[:], rhs2[:],
                                     start=True, stop=True)
                    h1 = npool.tile([HID, 256], bf16, tag="h1")
                    nc.scalar.activation(h1[:], ph[:], AF.Relu, bias=b2a_sb[:])
                    po = nppool.tile([NTGT, 256], f32, tag="po")
                    nc.tensor.matmul(po[:], W2b_sb[:], h1[:],
                                     start=True, stop=True)
                    ot = npool.tile([NTGT, 256], f32, tag="ot")
                    nc.scalar.activation(ot[:], po[:], AF.Identity,
                                         bias=b2b_sb[:])
                    nc.sync.dma_start(out=outT[:, G * 256:(G + 1) * 256],
                                      in_=ot[:])

    nc.compile()
    return nc


def _host_prep(x, edge_index, edge_attr, W1a, b1a, W1b, b1b, W2a, b2a,
               W2b, b2b):
    bf = ml_dtypes.bfloat16
    row = np.asarray(edge_index[0], dtype=np.int64)
    col = np.asarray(edge_index[1], dtype=np.int64)
    x = np.asarray(x, dtype=np.float32)
    ea = np.asarray(edge_attr, dtype=np.float32)

    xT_pad = np.zeros((F, HB * HB_N), bf)
    xT_pad[:, :N_NODES] = x.T

    I64e = np.zeros((128, HID + 1), np.float32)
    I64e[0:HID, 0:HID] = np.eye(HID)
    I64e[HID:128, 0:HID] = np.eye(HID)

    Btil = np.zeros((EAR, HID + 1), np.float32)
    Btil[0:F, 0:HID] = W1a[F:, :]
    Btil[F, 0:HID] = b1a
    Btil[F, HID] = 1.0

    # rhs2 row order: [Agg (64); ind (1); x (32)]
    W2a_eff = np.concatenate(
        [W2a[F:, :], (b1b @ W2a[F:, :]).reshape(1, HID), W2a[:F, :]],
        axis=0).astype(np.float32)

    iota = np.tile(np.arange(128, dtype=np.float32), (128, 1)).astype(bf)

    common = {
        "xT": xT_pad,
        "A_w": np.tile(np.asarray(W1a[:F, :], np.float32), (1, 2)).astype(bf),
        "I64_w": I64e.astype(bf),
        "Btil_w": Btil.astype(bf),
        "W1b_w": np.asarray(W1b, np.float32),
        "W2a_w": W2a_eff,
        "b2a_w": np.asarray(b2a, np.float32).reshape(HID, 1),
        "b2b_w": np.asarray(b2b, np.float32).reshape(NTGT, 1),
        "W2b_w": np.asarray(W2b, np.float32).astype(bf),
        "iota_w": iota,
        "idf_w": np.eye(128, dtype=np.float32),
    }

    shard = row // NSH
    in_maps = []
    for core in range(NCORES):
        sel = np.nonzero(shard == core)[0]
        lrow = (row[sel] - core * NSH).astype(np.int64)
        scol = col[sel]
        hb = scol // HB_N
        lcol = (scol - hb * HB_N).astype(np.int64)
        blk = lrow >> 7
        bid = hb * NBLK + blk
        order = np.lexsort((lrow, bid))
        sbid = bid[order]
        cnt = np.bincount(bid, minlength=NCELL_TOT)
        mx = cnt.max()
        assert mx <= CELL, f"cell overflow: {mx} > {CELL}"
        starts = np.zeros(NCELL_TOT + 1, np.int64)
        starts[1:] = np.cumsum(cnt)
        within = np.arange(len(order)) - starts[sbid]
        slots = sbid * CELL + within

        gidx = np.zeros(S_TOT, np.int16)          # poison -> node 0
        gidx[slots] = lcol[order].astype(np.int16)
        eaT = np.zeros((EAR, S_TOT), bf)
        eaT[0:F, slots] = ea[sel][order].T
        eaT[F, slots] = 1.0
        offs = np.full(S_TOT, OFFS_NONE, np.float32)
        offs[slots] = (lrow[order] & 127).astype(np.float32)

        # gather idx columns: col = slot, wrapped in 16 partitions and
        # replicated to all 8 q7 core groups
        gw = np.tile(gidx.reshape(S_TOT // 16, 16).T, (8, 1))

        x_ownT = np.zeros((F, NSH_PAD), np.float32)
        x_ownT[:, :NSH] = x[core * NSH:(core + 1) * NSH].T

        m = dict(common)
        m["x_ownT"] = x_ownT
        m["gidx_w"] = gw
        m["eaT_w"] = eaT
        m["offs_w"] = offs.reshape(S_TOT // 128, 128).T.astype(bf).copy()
        in_maps.append(m)
    return in_maps


def kernel(x, edge_index, edge_attr, u, batch,
           W1a, b1a, W1b, b1b, W2a, b2a, W2b, b2b, _profile=False):
    from concourse import bass_utils

    if "nc" not in _CACHE:
        _CACHE["nc"] = _build_nc()
    nc = _CACHE["nc"]

    in_maps = _host_prep(x, edge_index, edge_attr, W1a, b1a, W1b, b1b,
                         W2a, b2a, W2b, b2b)
    res = bass_utils.run_bass_kernel_spmd(
        nc, in_maps, core_ids=list(range(NCORES)), trace=_profile)
    out = np.empty((N_NODES, NTGT), np.float32)
    for core in range(NCORES):
        out[core * NSH:(core + 1) * NSH] = \
            res.results[core]["outT"][:, :NSH].T
    if _profile:
        _CACHE["last_exec_time_ns"] = res.exec_time_ns
    return out



# revision 6
# speedup vs baseline: 2.9706x; 2.6815x over previous
"""GNN message-passing (NodeModel) Trainium2 kernel, 8 NeuronCores.

Sharding: edges partitioned by destination node (12500 nodes/core) -> the
segment-sum stays core-local, no collectives. Per core, edges are grouped by
source half-bank (8 x 12544 nodes), sorted by destination, and laid out in
fixed 384-slot cells per (half-bank, 128-dest-block) so every core runs the
identical program (SPMD, one NEFF).

Math (W1b deferred out of the edge loop):
  z_e   = P[col_e] + ea_e @ B + b1a,   P = x @ W1a[:32]
  m_e   = relu(z_e);  S_n = sum_e m_e;  c_n = deg(n)
  agg_n = (S_n / max(c_n,1)) @ W1b + 1[c_n>0] * b1b
  out   = relu([x | agg] @ W2a + b2a) @ W2b + b2b

The per-edge P fetch avoids DMA gather entirely (random 256B HBM gathers
measured ~31 GB/s, latency-bound): P^T for one half-bank lives in SBUF
feature-major [64, 12544] f32 (computed on device), and gpsimd ap_gather
expands it per-slot along the free dim. Partitions 64:128 hold a copy of the
table so the 8 Q7 cores serve two slot-halves per gathered column (column c
-> rows 0:64 = slot c, rows 64:128 = slot c + CHUNK/2). z is accumulated in
PSUM from two matmuls (gathered-P against I64, edge attrs against Btil); S
and c are accumulated with one-hot matmuls into PSUM; b1b is folded into
W2a_eff with the count indicator as a 97th feature.
"""
import numpy as np
import ml_dtypes

N_NODES = 100000
F = 32
HID = 64
NTGT = 32
NCORES = 8

NSH = 12500            # dest nodes per core
NSH_PAD = 12544        # 98 * 128
NBLK = 98              # dest-node blocks (128 nodes) per core
HB = 8                 # source half-banks
HB_N = 12544           # source nodes per half-bank
CELL = 384             # slots per (half-bank, dest-block) = 3 tiles of 128
TPB = 3
NCELL_TOT = HB * NBLK              # 784
S_TOT = NCELL_TOT * CELL           # 301056
CHUNK_CELLS = 7
CHUNK = CHUNK_CELLS * CELL         # 2688 slots (= one gather call)
CHUNKS_PER_HB = NBLK // CHUNK_CELLS    # 14
HB_SLOTS = NBLK * CELL                 # 37632
OFFS_NONE = 200.0
GQ = 768               # gathered columns per chunk (4-way core-group split)
# chunk tile -> (quarter, col offset): quarters cover (6,5,5,5) tiles
QMAP = {}
for _t in range(21):
    if _t < 6:
        QMAP[_t] = (0, _t * 128)
    elif _t < 11:
        QMAP[_t] = (1, (_t - 6) * 128)
    elif _t < 16:
        QMAP[_t] = (2, (_t - 11) * 128)
    else:
        QMAP[_t] = (3, (_t - 16) * 128)
QBOUND = [(0, 768), (768, 1408), (1408, 2048), (2048, 2688)]
EAR = F + 1            # ea rows (32 feats + indicator)

_CACHE = {}


def _build_nc():
    import os
    import concourse.bass as bass
    import concourse.bacc as bacc
    import concourse.mybir as mybir
    from concourse.tile import TileContext
    from bass_rust import add_dep_helper

    f32 = mybir.dt.float32
    bf16 = mybir.dt.bfloat16
    i16 = mybir.dt.int16
    AF = mybir.ActivationFunctionType
    OP = mybir.AluOpType

    nc = bacc.Bacc("TRN2", target_bir_lowering=False, debug=False,
                   num_devices=NCORES)

    xT = nc.dram_tensor("xT", [F, HB * HB_N], bf16, kind="ExternalInput")
    x_ownT = nc.dram_tensor("x_ownT", [F, NSH_PAD], f32, kind="ExternalInput")
    A_w = nc.dram_tensor("A_w", [F, 128], bf16, kind="ExternalInput")
    I64_w = nc.dram_tensor("I64_w", [128, HID + 1], bf16, kind="ExternalInput")
    Btil_w = nc.dram_tensor("Btil_w", [EAR, HID + 1], bf16,
                            kind="ExternalInput")
    W1b_w = nc.dram_tensor("W1b_w", [HID, HID], f32, kind="ExternalInput")
    W2a_w = nc.dram_tensor("W2a_w", [F + HID + 1, HID], f32, kind="ExternalInput")
    b2a_w = nc.dram_tensor("b2a_w", [HID, 1], f32, kind="ExternalInput")
    W2b_w = nc.dram_tensor("W2b_w", [HID, NTGT], bf16, kind="ExternalInput")
    b2b_w = nc.dram_tensor("b2b_w", [NTGT, 1], f32, kind="ExternalInput")
    iota_w = nc.dram_tensor("iota_w", [128, 128], bf16, kind="ExternalInput")
    idf_w = nc.dram_tensor("idf_w", [128, 128], f32, kind="ExternalInput")
    gidx_w = nc.dram_tensor("gidx_w", [128, S_TOT // CHUNK * (GQ // 16)], i16,
                            kind="ExternalInput")
    eaT_w = nc.dram_tensor("eaT_w", [EAR, S_TOT], bf16, kind="ExternalInput")
    offs_w = nc.dram_tensor("offs_w", [128, S_TOT // 128], bf16,
                            kind="ExternalInput")
    outT = nc.dram_tensor("outT", [NTGT, NSH_PAD], f32, kind="ExternalOutput")

    with TileContext(nc) as tc:
        with tc.tile_pool(name="const", bufs=1) as cpool, \
             tc.tile_pool(name="acc", bufs=1) as apool, \
             tc.tile_pool(name="tbl", bufs=1) as tpool:

            # constants
            A_sb = cpool.tile([F, 128], bf16)
            nc.sync.dma_start(out=A_sb[:], in_=A_w[:])
            I64_sb = cpool.tile([128, HID + 1], bf16)
            nc.sync.dma_start(out=I64_sb[:], in_=I64_w[:])
            Btil_sb = cpool.tile([EAR, HID + 1], bf16)
            nc.sync.dma_start(out=Btil_sb[:], in_=Btil_w[:])
            W1b_sb = cpool.tile([HID, HID], f32)
            nc.sync.dma_start(out=W1b_sb[:], in_=W1b_w[:])
            W2a_sb = cpool.tile([F + HID + 1, HID], f32)
            nc.sync.dma_start(out=W2a_sb[:], in_=W2a_w[:])
            b2a_sb = cpool.tile([HID, 1], f32)
            nc.sync.dma_start(out=b2a_sb[:], in_=b2a_w[:])
            W2b_sb = cpool.tile([HID, NTGT], bf16)
            nc.sync.dma_start(out=W2b_sb[:], in_=W2b_w[:])
            b2b_sb = cpool.tile([NTGT, 1], f32)
            nc.sync.dma_start(out=b2b_sb[:], in_=b2b_w[:])
            iota_sb = cpool.tile([128, 128], bf16)
            nc.sync.dma_start(out=iota_sb[:], in_=iota_w[:])
            offs_sb = cpool.tile([128, S_TOT // 128], bf16)
            nc.sync.dma_start(out=offs_sb[:], in_=offs_w[:])
            idf = cpool.tile([128, 128], f32)
            nc.sync.dma_start(out=idf[:], in_=idf_w[:])

            acc_sb = apool.tile([128, NSH_PAD], f32)
            nc.vector.memset(acc_sb[:], 0.0)

            tbl = tpool.tile([128, HB_N, 1], f32)

            with tc.tile_pool(name="xtp", bufs=2) as xpool, \
                 tc.tile_pool(name="pbp", bufs=1, space="PSUM") as pbppool, \
                 tc.tile_pool(name="gidx", bufs=2) as gxpool, \
                 tc.tile_pool(name="gt", bufs=3) as gpool, \
                 tc.tile_pool(name="ea", bufs=2) as eapool, \
                 tc.tile_pool(name="msg", bufs=3) as mpool, \
                 tc.tile_pool(name="oh", bufs=3) as ohpool, \
                 tc.tile_pool(name="ebp", bufs=3, space="PSUM") as ebppool, \
                 tc.tile_pool(name="accp", bufs=2, space="PSUM") as accppool:

                for b in range(HB):
                    # ---- table build: P^T = A^T @ x^T, feature-major ----
                    for g0 in range(0, HB_N, 6272):
                        xT_sb = xpool.tile([F, 6272], bf16, tag="xT")
                        nc.sync.dma_start(
                            out=xT_sb[:],
                            in_=xT[:, b * HB_N + g0:b * HB_N + g0 + 6272])
                        for h0 in range(0, 6272, 512):
                            n = min(512, 6272 - h0)
                            ps = pbppool.tile([128, 512], f32, tag="pbp")
                            for q0 in range(0, n, 128):
                                nc.tensor.matmul(
                                    ps[:, q0:q0 + 128], A_sb[:],
                                    xT_sb[:, h0 + q0:h0 + q0 + 128],
                                    start=True, stop=True)
                            nc.scalar.activation(
                                tbl[:, g0 + h0:g0 + h0 + n, 0],
                                ps[:, 0:n], AF.Copy)

                    GHB = CHUNKS_PER_HB * (GQ // 16)
                    gidx_sb = gxpool.tile([128, GHB], i16, tag="gidx")
                    nc.sync.dma_start(
                        out=gidx_sb[:],
                        in_=gidx_w[:, b * GHB:(b + 1) * GHB])

                    if os.environ.get("SKIP_P2"):
                        continue
                    for c in range(CHUNKS_PER_HB):
                        gt = gpool.tile([128, GQ, 1], f32, tag="gt")
                        nc.gpsimd.ap_gather(
                            gt[:], tbl[:],
                            gidx_sb[:, c * (GQ // 16):
                                    (c + 1) * (GQ // 16)],
                            128, HB_N, 1, GQ)

                        comb = eapool.tile([F + EAR, CHUNK], bf16,
                                           tag="comb")
                        for q in range(4):
                            a0, a1 = QBOUND[q]
                            if (c + q) % 2 == 0:
                                nc.vector.tensor_copy(
                                    out=comb[0:F, a0:a1],
                                    in_=gt[32 * q:32 * q + 32, 0:a1 - a0, 0])
                            else:
                                nc.scalar.activation(
                                    comb[0:F, a0:a1],
                                    gt[32 * q:32 * q + 32, 0:a1 - a0, 0],
                                    AF.Copy)
                        slot0 = b * HB_SLOTS + c * CHUNK
                        nc.sync.dma_start(out=comb[F:F + EAR, :],
                                          in_=eaT_w[:, slot0:slot0 + CHUNK])

                        for ci in range(CHUNK_CELLS):
                            dblk = c * CHUNK_CELLS + ci
                            zb = ebppool.tile([128, TPB * (HID + 1)], f32,
                                              tag="ebp")
                            for t in range(TPB):
                                s = ci * CELL + t * 128       # chunk slot
                                nc.tensor.matmul(
                                    zb[:, t * (HID + 1):(t + 1) * (HID + 1)],
                                    comb[:, s:s + 128],
                                    I64_sb[0:F + EAR, :],
                                    start=True, stop=True)
                            zv = zb[:].rearrange("p (t e) -> p t e", e=HID + 1)
                            msg = mpool.tile([128, TPB, HID + 1], bf16,
                                             tag="msg")
                            nc.scalar.activation(msg[:], zv, AF.Relu)

                            oh = ohpool.tile([128, TPB, 128], bf16, tag="oh")
                            tcol = (b * NBLK + dblk) * TPB
                            o_ap = offs_sb[:, tcol:tcol + TPB]
                            o_b = bass.AP(o_ap.tensor, o_ap.offset,
                                          [o_ap.ap[0], o_ap.ap[1], [0, 128]])
                            i_ap = iota_sb[:]
                            i_b = bass.AP(i_ap.tensor, i_ap.offset,
                                          [i_ap.ap[0], [0, TPB], i_ap.ap[1]])
                            nc.vector.tensor_tensor(out=oh[:], in0=o_b,
                                                    in1=i_b, op=OP.is_equal)

                            if ci == 0:
                                accp = accppool.tile(
                                    [HID + 1, CHUNK_CELLS * 128], f32,
                                    tag="accp")
                            for t in range(TPB):
                                nc.tensor.matmul(
                                    accp[:, ci * 128:(ci + 1) * 128],
                                    msg[:, t, :], oh[:, t, :],
                                    start=(t == 0),
                                    stop=(t == TPB - 1))
                            if ci == CHUNK_CELLS - 1:
                                d0 = c * CHUNK_CELLS * 128
                                d1 = d0 + CHUNK_CELLS * 128
                                nc.vector.tensor_tensor(
                                    out=acc_sb[0:HID + 1, d0:d1],
                                    in0=acc_sb[0:HID + 1, d0:d1],
                                    in1=accp[:], op=OP.add)

            # ---- Phase 3: node MLP (streamed, 256 nodes per group) ----
            with tc.tile_pool(name="node", bufs=3) as npool, \
                 tc.tile_pool(name="nodep", bufs=1, space="PSUM") as nppool:
                for G in range(NSH_PAD // 256):         # 49 groups
                    # rows: 0:64 Agg^T, 64 ind^T, 65:97 x_own^T
                    rhs2 = npool.tile([F + HID + 1, 256], f32, tag="rhs2")
                    nc.sync.dma_start(out=rhs2[HID + 1:HID + 1 + F, :],
                                      in_=x_ownT[:, G * 256:(G + 1) * 256])
                    for j in range(2):
                        q = 2 * G + j
                        pS = nppool.tile([128, 128], f32, tag="pS")
                        nc.tensor.transpose(
                            out=pS[:], in_=acc_sb[:, q * 128:(q + 1) * 128],
                            identity=idf[:])
                        rec = npool.tile([128, 1], f32, tag="rec")
                        nc.vector.tensor_scalar_max(rec[:], pS[:, HID:HID + 1],
                                                    1.0)
                        nc.vector.reciprocal(rec[:], rec[:])
                        ind = npool.tile([128, 1], f32, tag="ind")
                        nc.vector.tensor_scalar_min(ind[:], pS[:, HID:HID + 1],
                                                    1.0)
                        pA = nppool.tile([128, HID], f32, tag="pA")
                        nc.tensor.matmul(pA[:],
                                         acc_sb[0:HID, q * 128:(q + 1) * 128],
                                         W1b_sb[:], start=True, stop=True)
                        tq = npool.tile([128, 128], f32, tag="tq")
                        nc.vector.tensor_scalar_mul(tq[:, 0:HID], pA[:], rec[:])
                        nc.vector.tensor_copy(out=tq[:, HID:HID + 1], in_=ind[:])
                        nc.vector.memset(tq[:, HID + 1:128], 0.0)
                        pT = nppool.tile([128, 128], f32, tag="pT")
                        nc.tensor.transpose(out=pT[:], in_=tq[:], identity=idf[:])
                        nc.vector.tensor_copy(
                            out=rhs2[0:HID + 1, j * 128:(j + 1) * 128],
                            in_=pT[0:HID + 1, :])
                    ph = nppool.tile([HID, 256], f32, tag="ph")
                    nc.tensor.matmul(ph[:], W2a_sb[:], rhs2[:],
                                     start=True, stop=True)
                    h1 = npool.tile([HID, 256], bf16, tag="h1")
                    nc.scalar.activation(h1[:], ph[:], AF.Relu, bias=b2a_sb[:])
                    po = nppool.tile([NTGT, 256], f32, tag="po")
                    nc.tensor.matmul(po[:], W2b_sb[:], h1[:],
                                     start=True, stop=True)
                    ot = npool.tile([NTGT, 256], f32, tag="ot")
                    nc.scalar.activation(ot[:], po[:], AF.Identity,
                                         bias=b2b_sb[:])
                    nc.sync.dma_start(out=outT[:, G * 256:(G + 1) * 256],
                                      in_=ot[:])

    nc.compile()
    return nc


def _host_prep(x, edge_index, edge_attr, W1a, b1a, W1b, b1b, W2a, b2a,
               W2b, b2b):
    bf = ml_dtypes.bfloat16
    row = np.asarray(edge_index[0], dtype=np.int64)
    col = np.asarray(edge_index[1], dtype=np.int64)
    x = np.asarray(x, dtype=np.float32)
    ea = np.asarray(edge_attr, dtype=np.float32)

    xT_pad = np.zeros((F, HB * HB_N), bf)
    xT_pad[:, :N_NODES] = x.T

    # merged z moving operand: rows 0:32 = W1a_top, rows 32:65 = Btil
    # (W1a ea-part + b1a row with count-indicator col)
    I64e = np.zeros((128, HID + 1), np.float32)
    I64e[0:F, 0:HID] = W1a[:F, :]
    I64e[F:F + F, 0:HID] = W1a[F:, :]
    I64e[2 * F, 0:HID] = b1a
    I64e[2 * F, HID] = 1.0

    Btil = np.zeros((EAR, HID + 1), np.float32)
    Btil[0:F, 0:HID] = W1a[F:, :]
    Btil[F, 0:HID] = b1a
    Btil[F, HID] = 1.0

    # rhs2 row order: [Agg (64); ind (1); x (32)]
    W2a_eff = np.concatenate(
        [W2a[F:, :], (b1b @ W2a[F:, :]).reshape(1, HID), W2a[:F, :]],
        axis=0).astype(np.float32)

    iota = np.tile(np.arange(128, dtype=np.float32), (128, 1)).astype(bf)

    common = {
        "xT": xT_pad,
        "A_w": np.tile(np.eye(F, dtype=np.float32), (1, 4)).astype(bf),
        "I64_w": I64e.astype(bf),
        "Btil_w": Btil.astype(bf),
        "W1b_w": np.asarray(W1b, np.float32),
        "W2a_w": W2a_eff,
        "b2a_w": np.asarray(b2a, np.float32).reshape(HID, 1),
        "b2b_w": np.asarray(b2b, np.float32).reshape(NTGT, 1),
        "W2b_w": np.asarray(W2b, np.float32).astype(bf),
        "iota_w": iota,
        "idf_w": np.eye(128, dtype=np.float32),
    }

    shard = row // NSH
    in_maps = []
    for core in range(NCORES):
        sel = np.nonzero(shard == core)[0]
        lrow = (row[sel] - core * NSH).astype(np.int64)
        scol = col[sel]
        hb = scol // HB_N
        lcol = (scol - hb * HB_N).astype(np.int64)
        blk = lrow >> 7
        bid = hb * NBLK + blk
        order = np.lexsort((lrow, bid))
        sbid = bid[order]
        cnt = np.bincount(bid, minlength=NCELL_TOT)
        mx = cnt.max()
        assert mx <= CELL, f"cell overflow: {mx} > {CELL}"
        starts = np.zeros(NCELL_TOT + 1, np.int64)
        starts[1:] = np.cumsum(cnt)
        within = np.arange(len(order)) - starts[sbid]
        slots = sbid * CELL + within

        gidx = np.zeros(S_TOT, np.int16)          # poison -> node 0
        gidx[slots] = lcol[order].astype(np.int16)
        eaT = np.zeros((EAR, S_TOT), bf)
        eaT[0:F, slots] = ea[sel][order].T
        eaT[F, slots] = 1.0
        offs = np.full(S_TOT, OFFS_NONE, np.float32)
        offs[slots] = (lrow[order] & 127).astype(np.float32)

        # gather idx columns, 4-way core-group split: quarter q's Q7 pair
        # (partitions 32q:32q+32) serves chunk slots QBOUND[q], padded to GQ
        nchunks = S_TOT // CHUNK
        gw = np.zeros((128, nchunks * (GQ // 16)), np.int16)
        gcv = gidx.reshape(nchunks, CHUNK)
        for ck in range(nchunks):
            for q, (a0, a1) in enumerate(QBOUND):
                qi = np.zeros(GQ, np.int16)
                qi[:a1 - a0] = gcv[ck, a0:a1]
                w = qi.reshape(GQ // 16, 16).T        # [16, 48]
                gw[32 * q:32 * q + 32,
                   ck * (GQ // 16):(ck + 1) * (GQ // 16)] = np.tile(w, (2, 1))

        x_ownT = np.zeros((F, NSH_PAD), np.float32)
        x_ownT[:, :NSH] = x[core * NSH:(core + 1) * NSH].T

        m = dict(common)
        m["x_ownT"] = x_ownT
        m["gidx_w"] = gw
        m["eaT_w"] = eaT
        m["offs_w"] = offs.reshape(S_TOT // 128, 128).T.astype(bf).copy()
        in_maps.append(m)
    return in_maps


def kernel(x, edge_index, edge_attr, u, batch,
           W1a, b1a, W1b, b1b, W2a, b2a, W2b, b2b, _profile=False):
    from concourse import bass_utils

    if "nc" not in _CACHE:
        _CACHE["nc"] = _build_nc()
    nc = _CACHE["nc"]

    in_maps = _host_prep(x, edge_index, edge_attr, W1a, b1a, W1b, b1b,
                         W2a, b2a, W2b, b2b)
    import os as _os
    if _os.environ.get("BASS_SIM"):
        from concourse.bass_interp import CoreSim
        sim = CoreSim(nc, trace=False)
        for name, arr in in_maps[0].items():
            sim.tensor(name)[:] = arr
        sim.simulate()
        outT = np.asarray(sim.tensor("outT"))
        out = np.zeros((N_NODES, NTGT), np.float32)
        out[:NSH] = outT[:, :NSH].T
        return out
    res = bass_utils.run_bass_kernel_spmd(
        nc, in_maps, core_ids=list(range(NCORES)), trace=_profile)
    out = np.empty((N_NODES, NTGT), np.float32)
    for core in range(NCORES):
        out[core * NSH:(core + 1) * NSH] = \
            res.results[core]["outT"][:, :NSH].T
    if _profile:
        _CACHE["last_exec_time_ns"] = res.exec_time_ns
    return out



# revision 7
# speedup vs baseline: 2.9770x; 1.0022x over previous
"""GNN message-passing (NodeModel) Trainium2 kernel, 8 NeuronCores.

Sharding: edges partitioned by destination node (12500 nodes/core) -> the
segment-sum stays core-local, no collectives. Per core, edges are grouped by
source half-bank (8 x 12544 nodes), sorted by destination, and laid out in
fixed 384-slot cells per (half-bank, 128-dest-block) so every core runs the
identical program (SPMD, one NEFF).

Math (W1b deferred out of the edge loop):
  z_e   = P[col_e] + ea_e @ B + b1a,   P = x @ W1a[:32]
  m_e   = relu(z_e);  S_n = sum_e m_e;  c_n = deg(n)
  agg_n = (S_n / max(c_n,1)) @ W1b + 1[c_n>0] * b1b
  out   = relu([x | agg] @ W2a + b2a) @ W2b + b2b

The per-edge P fetch avoids DMA gather entirely (random 256B HBM gathers
measured ~31 GB/s, latency-bound): P^T for one half-bank lives in SBUF
feature-major [64, 12544] f32 (computed on device), and gpsimd ap_gather
expands it per-slot along the free dim. Partitions 64:128 hold a copy of the
table so the 8 Q7 cores serve two slot-halves per gathered column (column c
-> rows 0:64 = slot c, rows 64:128 = slot c + CHUNK/2). z is accumulated in
PSUM from two matmuls (gathered-P against I64, edge attrs against Btil); S
and c are accumulated with one-hot matmuls into PSUM; b1b is folded into
W2a_eff with the count indicator as a 97th feature.
"""
import numpy as np
import ml_dtypes

N_NODES = 100000
F = 32
HID = 64
NTGT = 32
NCORES = 8

NSH = 12500            # dest nodes per core
NSH_PAD = 12544        # 98 * 128
NBLK = 98              # dest-node blocks (128 nodes) per core
HB = 8                 # source half-banks
HB_N = 12544           # source nodes per half-bank
CELL = 384             # slots per (half-bank, dest-block) = 3 tiles of 128
TPB = 3
NCELL_TOT = HB * NBLK              # 784
S_TOT = NCELL_TOT * CELL           # 301056
CHUNK_CELLS = 7
CHUNK = CHUNK_CELLS * CELL         # 2688 slots (= one gather call)
CHUNKS_PER_HB = NBLK // CHUNK_CELLS    # 14
HB_SLOTS = NBLK * CELL                 # 37632
OFFS_NONE = 200.0
GQ = 768               # gathered columns per chunk (4-way core-group split)
# chunk tile -> (quarter, col offset): quarters cover (6,5,5,5) tiles
QMAP = {}
for _t in range(21):
    if _t < 6:
        QMAP[_t] = (0, _t * 128)
    elif _t < 11:
        QMAP[_t] = (1, (_t - 6) * 128)
    elif _t < 16:
        QMAP[_t] = (2, (_t - 11) * 128)
    else:
        QMAP[_t] = (3, (_t - 16) * 128)
QBOUND = [(0, 768), (768, 1408), (1408, 2048), (2048, 2688)]
EAR = F + 1            # ea rows (32 feats + indicator)

_CACHE = {}


def _build_nc():
    import os
    import concourse.bass as bass
    import concourse.bacc as bacc
    import concourse.mybir as mybir
    from concourse.tile import TileContext
    from bass_rust import add_dep_helper

    f32 = mybir.dt.float32
    bf16 = mybir.dt.bfloat16
    i16 = mybir.dt.int16
    AF = mybir.ActivationFunctionType
    OP = mybir.AluOpType

    nc = bacc.Bacc("TRN2", target_bir_lowering=False, debug=False,
                   num_devices=NCORES)

    xT = nc.dram_tensor("xT", [F, HB * HB_N], bf16, kind="ExternalInput")
    x_ownT = nc.dram_tensor("x_ownT", [F, NSH_PAD], f32, kind="ExternalInput")
    A_w = nc.dram_tensor("A_w", [F, 128], bf16, kind="ExternalInput")
    I64_w = nc.dram_tensor("I64_w", [128, HID + 1], bf16, kind="ExternalInput")
    Btil_w = nc.dram_tensor("Btil_w", [EAR, HID + 1], bf16,
                            kind="ExternalInput")
    W1b_w = nc.dram_tensor("W1b_w", [HID, HID], f32, kind="ExternalInput")
    W2a_w = nc.dram_tensor("W2a_w", [F + HID + 1, HID], f32, kind="ExternalInput")
    b2a_w = nc.dram_tensor("b2a_w", [HID, 1], f32, kind="ExternalInput")
    W2b_w = nc.dram_tensor("W2b_w", [HID, NTGT], bf16, kind="ExternalInput")
    b2b_w = nc.dram_tensor("b2b_w", [NTGT, 1], f32, kind="ExternalInput")
    iota_w = nc.dram_tensor("iota_w", [128, 128], bf16, kind="ExternalInput")
    idf_w = nc.dram_tensor("idf_w", [128, 128], f32, kind="ExternalInput")
    gidx_w = nc.dram_tensor("gidx_w", [128, S_TOT // CHUNK * (GQ // 16)], i16,
                            kind="ExternalInput")
    eaT_w = nc.dram_tensor("eaT_w", [EAR, S_TOT], bf16, kind="ExternalInput")
    offs_w = nc.dram_tensor("offs_w", [128, S_TOT // 128], bf16,
                            kind="ExternalInput")
    outT = nc.dram_tensor("outT", [NTGT, NSH_PAD], f32, kind="ExternalOutput")

    with TileContext(nc) as tc:
        with tc.tile_pool(name="const", bufs=1) as cpool, \
             tc.tile_pool(name="acc", bufs=1) as apool, \
             tc.tile_pool(name="tbl", bufs=1) as tpool:

            # constants
            A_sb = cpool.tile([F, 128], bf16)
            nc.sync.dma_start(out=A_sb[:], in_=A_w[:])
            I64_sb = cpool.tile([128, HID + 1], bf16)
            nc.sync.dma_start(out=I64_sb[:], in_=I64_w[:])
            Btil_sb = cpool.tile([EAR, HID + 1], bf16)
            nc.sync.dma_start(out=Btil_sb[:], in_=Btil_w[:])
            W1b_sb = cpool.tile([HID, HID], f32)
            nc.sync.dma_start(out=W1b_sb[:], in_=W1b_w[:])
            W2a_sb = cpool.tile([F + HID + 1, HID], f32)
            nc.sync.dma_start(out=W2a_sb[:], in_=W2a_w[:])
            b2a_sb = cpool.tile([HID, 1], f32)
            nc.sync.dma_start(out=b2a_sb[:], in_=b2a_w[:])
            W2b_sb = cpool.tile([HID, NTGT], bf16)
            nc.sync.dma_start(out=W2b_sb[:], in_=W2b_w[:])
            b2b_sb = cpool.tile([NTGT, 1], f32)
            nc.sync.dma_start(out=b2b_sb[:], in_=b2b_w[:])
            iota_sb = cpool.tile([128, 128], bf16)
            nc.sync.dma_start(out=iota_sb[:], in_=iota_w[:])
            offs_sb = cpool.tile([128, S_TOT // 128], bf16)
            nc.sync.dma_start(out=offs_sb[:], in_=offs_w[:])
            idf = cpool.tile([128, 128], f32)
            nc.sync.dma_start(out=idf[:], in_=idf_w[:])

            acc_sb = apool.tile([128, NSH_PAD], f32)
            nc.vector.memset(acc_sb[:], 0.0)

            tbl = tpool.tile([128, HB_N, 1], f32)

            with tc.tile_pool(name="xtp", bufs=2) as xpool, \
                 tc.tile_pool(name="pbp", bufs=1, space="PSUM") as pbppool, \
                 tc.tile_pool(name="gidx", bufs=2) as gxpool, \
                 tc.tile_pool(name="gt", bufs=6) as gpool, \
                 tc.tile_pool(name="ea", bufs=4) as eapool, \
                 tc.tile_pool(name="msg", bufs=4) as mpool, \
                 tc.tile_pool(name="oh", bufs=4) as ohpool, \
                 tc.tile_pool(name="ebp", bufs=3, space="PSUM") as ebppool, \
                 tc.tile_pool(name="accp", bufs=2, space="PSUM") as accppool:

                for b in range(HB):
                    # ---- table build: P^T = A^T @ x^T, feature-major ----
                    for g0 in range(0, HB_N, 6272):
                        xT_sb = xpool.tile([F, 6272], bf16, tag="xT")
                        nc.sync.dma_start(
                            out=xT_sb[:],
                            in_=xT[:, b * HB_N + g0:b * HB_N + g0 + 6272])
                        for h0 in range(0, 6272, 512):
                            n = min(512, 6272 - h0)
                            ps = pbppool.tile([128, 512], f32, tag="pbp")
                            for q0 in range(0, n, 128):
                                nc.tensor.matmul(
                                    ps[:, q0:q0 + 128], A_sb[:],
                                    xT_sb[:, h0 + q0:h0 + q0 + 128],
                                    start=True, stop=True)
                            nc.scalar.activation(
                                tbl[:, g0 + h0:g0 + h0 + n, 0],
                                ps[:, 0:n], AF.Copy)

                    GHB = CHUNKS_PER_HB * (GQ // 16)
                    gidx_sb = gxpool.tile([128, GHB], i16, tag="gidx")
                    nc.sync.dma_start(
                        out=gidx_sb[:],
                        in_=gidx_w[:, b * GHB:(b + 1) * GHB])

                    if os.environ.get("SKIP_P2"):
                        continue
                    for c in range(CHUNKS_PER_HB):
                        gt = gpool.tile([128, GQ, 1], f32, tag="gt")
                        nc.gpsimd.ap_gather(
                            gt[:], tbl[:],
                            gidx_sb[:, c * (GQ // 16):
                                    (c + 1) * (GQ // 16)],
                            128, HB_N, 1, GQ)

                        comb = eapool.tile([F + EAR, CHUNK], bf16,
                                           tag="comb")
                        for q in range(4):
                            a0, a1 = QBOUND[q]
                            if (c + q) % 2 == 0:
                                nc.vector.tensor_copy(
                                    out=comb[0:F, a0:a1],
                                    in_=gt[32 * q:32 * q + 32, 0:a1 - a0, 0])
                            else:
                                nc.scalar.activation(
                                    comb[0:F, a0:a1],
                                    gt[32 * q:32 * q + 32, 0:a1 - a0, 0],
                                    AF.Copy)
                        slot0 = b * HB_SLOTS + c * CHUNK
                        nc.sync.dma_start(out=comb[F:F + EAR, :],
                                          in_=eaT_w[:, slot0:slot0 + CHUNK])

                        for ci in range(CHUNK_CELLS):
                            dblk = c * CHUNK_CELLS + ci
                            zb = ebppool.tile([128, TPB * (HID + 1)], f32,
                                              tag="ebp")
                            for t in range(TPB):
                                s = ci * CELL + t * 128       # chunk slot
                                nc.tensor.matmul(
                                    zb[:, t * (HID + 1):(t + 1) * (HID + 1)],
                                    comb[:, s:s + 128],
                                    I64_sb[0:F + EAR, :],
                                    start=True, stop=True)
                            zv = zb[:].rearrange("p (t e) -> p t e", e=HID + 1)
                            msg = mpool.tile([128, TPB, HID + 1], bf16,
                                             tag="msg")
                            nc.scalar.activation(msg[:], zv, AF.Relu)

                            oh = ohpool.tile([128, TPB, 128], bf16, tag="oh")
                            tcol = (b * NBLK + dblk) * TPB
                            o_ap = offs_sb[:, tcol:tcol + TPB]
                            o_b = bass.AP(o_ap.tensor, o_ap.offset,
                                          [o_ap.ap[0], o_ap.ap[1], [0, 128]])
                            i_ap = iota_sb[:]
                            i_b = bass.AP(i_ap.tensor, i_ap.offset,
                                          [i_ap.ap[0], [0, TPB], i_ap.ap[1]])
                            nc.vector.tensor_tensor(out=oh[:], in0=o_b,
                                                    in1=i_b, op=OP.is_equal)

                            if ci == 0:
                                accp = accppool.tile(
                                    [HID + 1, CHUNK_CELLS * 128], f32,
                                    tag="accp")
                            for t in range(TPB):
                                nc.tensor.matmul(
                                    accp[:, ci * 128:(ci + 1) * 128],
                                    msg[:, t, :], oh[:, t, :],
                                    start=(t == 0),
                                    stop=(t == TPB - 1))
                            if ci == CHUNK_CELLS - 1:
                                d0 = c * CHUNK_CELLS * 128
                                d1 = d0 + CHUNK_CELLS * 128
                                nc.vector.tensor_tensor(
                                    out=acc_sb[0:HID + 1, d0:d1],
                                    in0=acc_sb[0:HID + 1, d0:d1],
                                    in1=accp[:], op=OP.add)

            # ---- Phase 3: node MLP (streamed, 256 nodes per group) ----
            with tc.tile_pool(name="node", bufs=3) as npool, \
                 tc.tile_pool(name="nodep", bufs=1, space="PSUM") as nppool:
                for G in range(NSH_PAD // 256):         # 49 groups
                    # rows: 0:64 Agg^T, 64 ind^T, 65:97 x_own^T
                    rhs2 = npool.tile([F + HID + 1, 256], f32, tag="rhs2")
                    nc.sync.dma_start(out=rhs2[HID + 1:HID + 1 + F, :],
                                      in_=x_ownT[:, G * 256:(G + 1) * 256])
                    for j in range(2):
                        q = 2 * G + j
                        pS = nppool.tile([128, 128], f32, tag="pS")
                        nc.tensor.transpose(
                            out=pS[:], in_=acc_sb[:, q * 128:(q + 1) * 128],
                            identity=idf[:])
                        rec = npool.tile([128, 1], f32, tag="rec")
                        nc.vector.tensor_scalar_max(rec[:], pS[:, HID:HID + 1],
                                                    1.0)
                        nc.vector.reciprocal(rec[:], rec[:])
                        ind = npool.tile([128, 1], f32, tag="ind")
                        nc.vector.tensor_scalar_min(ind[:], pS[:, HID:HID + 1],
                                                    1.0)
                        pA = nppool.tile([128, HID], f32, tag="pA")
                        nc.tensor.matmul(pA[:],
                                         acc_sb[0:HID, q * 128:(q + 1) * 128],
                                         W1b_sb[:], start=True, stop=True)
                        tq = npool.tile([128, 128], f32, tag="tq")
                        nc.vector.tensor_scalar_mul(tq[:, 0:HID], pA[:], rec[:])
                        nc.vector.tensor_copy(out=tq[:, HID:HID + 1], in_=ind[:])
                        nc.vector.memset(tq[:, HID + 1:128], 0.0)
                        pT = nppool.tile([128, 128], f32, tag="pT")
                        nc.tensor.transpose(out=pT[:], in_=tq[:], identity=idf[:])
                        nc.vector.tensor_copy(
                            out=rhs2[0:HID + 1, j * 128:(j + 1) * 128],
                            in_=pT[0:HID + 1, :])
                    ph = nppool.tile([HID, 256], f32, tag="ph")
                    nc.tensor.matmul(ph[:], W2a_sb[:], rhs2[:],
                                     start=True, stop=True)
                    h1 = npool.tile([HID, 256], bf16, tag="h1")
                    nc.scalar.activation(h1[:], ph[:], AF.Relu, bias=b2a_sb[:])
                    po = nppool.tile([NTGT, 256], f32, tag="po")
                    nc.tensor.matmul(po[:], W2b_sb[:], h1[:],
                                     start=True, stop=True)
                    ot = npool.tile([NTGT, 256], f32, tag="ot")
                    nc.scalar.activation(ot[:], po[:], AF.Identity,
                                         bias=b2b_sb[:])
                    nc.sync.dma_start(out=outT[:, G * 256:(G + 1) * 256],
                                      in_=ot[:])

    nc.compile()
    return nc


def _host_prep(x, edge_index, edge_attr, W1a, b1a, W1b, b1b, W2a, b2a,
               W2b, b2b):
    bf = ml_dtypes.bfloat16
    row = np.asarray(edge_index[0], dtype=np.int64)
    col = np.asarray(edge_index[1], dtype=np.int64)
    x = np.asarray(x, dtype=np.float32)
    ea = np.asarray(edge_attr, dtype=np.float32)

    xT_pad = np.zeros((F, HB * HB_N), bf)
    xT_pad[:, :N_NODES] = x.T

    # merged z moving operand: rows 0:32 = W1a_top, rows 32:65 = Btil
    # (W1a ea-part + b1a row with count-indicator col)
    I64e = np.zeros((128, HID + 1), np.float32)
    I64e[0:F, 0:HID] = W1a[:F, :]
    I64e[F:F + F, 0:HID] = W1a[F:, :]
    I64e[2 * F, 0:HID] = b1a
    I64e[2 * F, HID] = 1.0

    Btil = np.zeros((EAR, HID + 1), np.float32)
    Btil[0:F, 0:HID] = W1a[F:, :]
    Btil[F, 0:HID] = b1a
    Btil[F, HID] = 1.0

    # rhs2 row order: [Agg (64); ind (1); x (32)]
    W2a_eff = np.concatenate(
        [W2a[F:, :], (b1b @ W2a[F:, :]).reshape(1, HID), W2a[:F, :]],
        axis=0).astype(np.float32)

    iota = np.tile(np.arange(128, dtype=np.float32), (128, 1)).astype(bf)

    common = {
        "xT": xT_pad,
        "A_w": np.tile(np.eye(F, dtype=np.float32), (1, 4)).astype(bf),
        "I64_w": I64e.astype(bf),
        "Btil_w": Btil.astype(bf),
        "W1b_w": np.asarray(W1b, np.float32),
        "W2a_w": W2a_eff,
        "b2a_w": np.asarray(b2a, np.float32).reshape(HID, 1),
        "b2b_w": np.asarray(b2b, np.float32).reshape(NTGT, 1),
        "W2b_w": np.asarray(W2b, np.float32).astype(bf),
        "iota_w": iota,
        "idf_w": np.eye(128, dtype=np.float32),
    }

    shard = row // NSH
    in_maps = []
    for core in range(NCORES):
        sel = np.nonzero(shard == core)[0]
        lrow = (row[sel] - core * NSH).astype(np.int64)
        scol = col[sel]
        hb = scol // HB_N
        lcol = (scol - hb * HB_N).astype(np.int64)
        blk = lrow >> 7
        bid = hb * NBLK + blk
        order = np.lexsort((lrow, bid))
        sbid = bid[order]
        cnt = np.bincount(bid, minlength=NCELL_TOT)
        mx = cnt.max()
        assert mx <= CELL, f"cell overflow: {mx} > {CELL}"
        starts = np.zeros(NCELL_TOT + 1, np.int64)
        starts[1:] = np.cumsum(cnt)
        within = np.arange(len(order)) - starts[sbid]
        slots = sbid * CELL + within

        gidx = np.zeros(S_TOT, np.int16)          # poison -> node 0
        gidx[slots] = lcol[order].astype(np.int16)
        eaT = np.zeros((EAR, S_TOT), bf)
        eaT[0:F, slots] = ea[sel][order].T
        eaT[F, slots] = 1.0
        offs = np.full(S_TOT, OFFS_NONE, np.float32)
        offs[slots] = (lrow[order] & 127).astype(np.float32)

        # gather idx columns, 4-way core-group split: quarter q's Q7 pair
        # (partitions 32q:32q+32) serves chunk slots QBOUND[q], padded to GQ
        nchunks = S_TOT // CHUNK
        gw = np.zeros((128, nchunks * (GQ // 16)), np.int16)
        gcv = gidx.reshape(nchunks, CHUNK)
        for ck in range(nchunks):
            for q, (a0, a1) in enumerate(QBOUND):
                qi = np.zeros(GQ, np.int16)
                qi[:a1 - a0] = gcv[ck, a0:a1]
                w = qi.reshape(GQ // 16, 16).T        # [16, 48]
                gw[32 * q:32 * q + 32,
                   ck * (GQ // 16):(ck + 1) * (GQ // 16)] = np.tile(w, (2, 1))

        x_ownT = np.zeros((F, NSH_PAD), np.float32)
        x_ownT[:, :NSH] = x[core * NSH:(core + 1) * NSH].T

        m = dict(common)
        m["x_ownT"] = x_ownT
        m["gidx_w"] = gw
        m["eaT_w"] = eaT
        m["offs_w"] = offs.reshape(S_TOT // 128, 128).T.astype(bf).copy()
        in_maps.append(m)
    return in_maps


def kernel(x, edge_index, edge_attr, u, batch,
           W1a, b1a, W1b, b1b, W2a, b2a, W2b, b2b, _profile=False):
    from concourse import bass_utils

    if "nc" not in _CACHE:
        _CACHE["nc"] = _build_nc()
    nc = _CACHE["nc"]

    in_maps = _host_prep(x, edge_index, edge_attr, W1a, b1a, W1b, b1b,
                         W2a, b2a, W2b, b2b)
    import os as _os
    if _os.environ.get("BASS_SIM"):
        from concourse.bass_interp import CoreSim
        sim = CoreSim(nc, trace=False)
        for name, arr in in_maps[0].items():
            sim.tensor(name)[:] = arr
        sim.simulate()
        outT = np.asarray(sim.tensor("outT"))
        out = np.zeros((N_NODES, NTGT), np.float32)
        out[:NSH] = outT[:, :NSH].T
        return out
    res = bass_utils.run_bass_kernel_spmd(
        nc, in_maps, core_ids=list(range(NCORES)), trace=_profile)
    out = np.empty((N_NODES, NTGT), np.float32)
    for core in range(NCORES):
        out[core * NSH:(core + 1) * NSH] = \
            res.results[core]["outT"][:, :NSH].T
    if _profile:
        _CACHE["last_exec_time_ns"] = res.exec_time_ns
    return out



# revision 8
# speedup vs baseline: 3.2025x; 1.0757x over previous
"""GNN message-passing (NodeModel) Trainium2 kernel, 8 NeuronCores.

Sharding: edges partitioned by destination node (12500 nodes/core) -> the
segment-sum stays core-local, no collectives. Per core, edges are grouped by
source half-bank (8 x 12544 nodes), sorted by destination, and laid out in
fixed 384-slot cells per (half-bank, 128-dest-block) so every core runs the
identical program (SPMD, one NEFF).

Math (W1b deferred out of the edge loop):
  z_e   = P[col_e] + ea_e @ B + b1a,   P = x @ W1a[:32]
  m_e   = relu(z_e);  S_n = sum_e m_e;  c_n = deg(n)
  agg_n = (S_n / max(c_n,1)) @ W1b + 1[c_n>0] * b1b
  out   = relu([x | agg] @ W2a + b2a) @ W2b + b2b

The per-edge P fetch avoids DMA gather entirely (random 256B HBM gathers
measured ~31 GB/s, latency-bound): P^T for one half-bank lives in SBUF
feature-major [64, 12544] f32 (computed on device), and gpsimd ap_gather
expands it per-slot along the free dim. Partitions 64:128 hold a copy of the
table so the 8 Q7 cores serve two slot-halves per gathered column (column c
-> rows 0:64 = slot c, rows 64:128 = slot c + CHUNK/2). z is accumulated in
PSUM from two matmuls (gathered-P against I64, edge attrs against Btil); S
and c are accumulated with one-hot matmuls into PSUM; b1b is folded into
W2a_eff with the count indicator as a 97th feature.
"""
import numpy as np
import ml_dtypes

N_NODES = 100000
F = 32
HID = 64
NTGT = 32
NCORES = 8

NSH = 12500            # dest nodes per core
NSH_PAD = 12544        # 98 * 128
NBLK = 98              # dest-node blocks (128 nodes) per core
HB = 8                 # source half-banks
HB_N = 12544           # source nodes per half-bank
CELL = 384             # slots per (half-bank, dest-block) = 3 tiles of 128
TPB = 3
NCELL_TOT = HB * NBLK              # 784
S_TOT = NCELL_TOT * CELL           # 301056
CHUNK_CELLS = 7
CHUNK = CHUNK_CELLS * CELL         # 2688 slots (= one gather call)
CHUNKS_PER_HB = NBLK // CHUNK_CELLS    # 14
HB_SLOTS = NBLK * CELL                 # 37632
OFFS_NONE = 200.0
GQ = 768               # gathered columns per chunk (4-way core-group split)
# chunk tile -> (quarter, col offset): quarters cover (6,5,5,5) tiles
QMAP = {}
for _t in range(21):
    if _t < 6:
        QMAP[_t] = (0, _t * 128)
    elif _t < 11:
        QMAP[_t] = (1, (_t - 6) * 128)
    elif _t < 16:
        QMAP[_t] = (2, (_t - 11) * 128)
    else:
        QMAP[_t] = (3, (_t - 16) * 128)
QBOUND = [(0, 768), (768, 1408), (1408, 2048), (2048, 2688)]
EAR = F + 1            # ea rows (32 feats + indicator)

_CACHE = {}


def _build_nc():
    import os
    import concourse.bass as bass
    import concourse.bacc as bacc
    import concourse.mybir as mybir
    from concourse.tile import TileContext
    from bass_rust import add_dep_helper

    f32 = mybir.dt.float32
    bf16 = mybir.dt.bfloat16
    i16 = mybir.dt.int16
    AF = mybir.ActivationFunctionType
    OP = mybir.AluOpType

    nc = bacc.Bacc("TRN2", target_bir_lowering=False, debug=False,
                   num_devices=NCORES)

    xT = nc.dram_tensor("xT", [F, HB * HB_N], bf16, kind="ExternalInput")
    x_ownT = nc.dram_tensor("x_ownT", [F, NSH_PAD], f32, kind="ExternalInput")
    A_w = nc.dram_tensor("A_w", [F, 128], bf16, kind="ExternalInput")
    I64_w = nc.dram_tensor("I64_w", [128, HID + 1], bf16, kind="ExternalInput")
    Btil_w = nc.dram_tensor("Btil_w", [EAR, HID + 1], bf16,
                            kind="ExternalInput")
    W1b_w = nc.dram_tensor("W1b_w", [HID, HID], f32, kind="ExternalInput")
    W2a_w = nc.dram_tensor("W2a_w", [F + HID + 1, HID], f32, kind="ExternalInput")
    b2a_w = nc.dram_tensor("b2a_w", [HID, 1], f32, kind="ExternalInput")
    W2b_w = nc.dram_tensor("W2b_w", [HID, NTGT], bf16, kind="ExternalInput")
    b2b_w = nc.dram_tensor("b2b_w", [NTGT, 1], f32, kind="ExternalInput")
    iota_w = nc.dram_tensor("iota_w", [128, 128], bf16, kind="ExternalInput")
    idf_w = nc.dram_tensor("idf_w", [128, 128], f32, kind="ExternalInput")
    gidx_w = nc.dram_tensor("gidx_w", [128, S_TOT // CHUNK * (GQ // 16)], i16,
                            kind="ExternalInput")
    eaT_w = nc.dram_tensor("eaT_w", [EAR, S_TOT], bf16, kind="ExternalInput")
    offs_w = nc.dram_tensor("offs_w", [128, S_TOT // 128], bf16,
                            kind="ExternalInput")
    outT = nc.dram_tensor("outT", [NTGT, NSH_PAD], f32, kind="ExternalOutput")

    with TileContext(nc) as tc:
        with tc.tile_pool(name="const", bufs=1) as cpool, \
             tc.tile_pool(name="acc", bufs=1) as apool, \
             tc.tile_pool(name="tbl", bufs=2) as tpool:

            # constants
            A_sb = cpool.tile([F, 128], bf16)
            nc.sync.dma_start(out=A_sb[:], in_=A_w[:])
            I64_sb = cpool.tile([128, HID + 1], bf16)
            nc.sync.dma_start(out=I64_sb[:], in_=I64_w[:])
            Btil_sb = cpool.tile([EAR, HID + 1], bf16)
            nc.sync.dma_start(out=Btil_sb[:], in_=Btil_w[:])
            W1b_sb = cpool.tile([HID, HID], f32)
            nc.sync.dma_start(out=W1b_sb[:], in_=W1b_w[:])
            W2a_sb = cpool.tile([F + HID + 1, HID], f32)
            nc.sync.dma_start(out=W2a_sb[:], in_=W2a_w[:])
            b2a_sb = cpool.tile([HID, 1], f32)
            nc.sync.dma_start(out=b2a_sb[:], in_=b2a_w[:])
            W2b_sb = cpool.tile([HID, NTGT], bf16)
            nc.sync.dma_start(out=W2b_sb[:], in_=W2b_w[:])
            b2b_sb = cpool.tile([NTGT, 1], f32)
            nc.sync.dma_start(out=b2b_sb[:], in_=b2b_w[:])
            iota_sb = cpool.tile([128, 128], bf16)
            nc.sync.dma_start(out=iota_sb[:], in_=iota_w[:])
            offs_sb = cpool.tile([128, S_TOT // 128], bf16)
            nc.sync.dma_start(out=offs_sb[:], in_=offs_w[:])
            idf = cpool.tile([128, 128], f32)
            nc.sync.dma_start(out=idf[:], in_=idf_w[:])

            acc_sb = apool.tile([128, NSH_PAD], f32)
            nc.vector.memset(acc_sb[:], 0.0)

            with tc.tile_pool(name="xtp", bufs=1) as xpool, \
                 tc.tile_pool(name="pbp", bufs=1, space="PSUM") as pbppool, \
                 tc.tile_pool(name="gidx", bufs=2) as gxpool, \
                 tc.tile_pool(name="gt", bufs=3) as gpool, \
                 tc.tile_pool(name="ea", bufs=2) as eapool, \
                 tc.tile_pool(name="msg", bufs=4) as mpool, \
                 tc.tile_pool(name="oh", bufs=4) as ohpool, \
                 tc.tile_pool(name="ebp", bufs=3, space="PSUM") as ebppool, \
                 tc.tile_pool(name="accp", bufs=2, space="PSUM") as accppool:

                for b in range(HB):
                    tbl = tpool.tile([128, HB_N, 1], f32, tag="tbl")
                    # ---- table build: x^T replicated 4x, feature-major ----
                    for g0 in range(0, HB_N, 6272):
                        xT_sb = xpool.tile([F, 6272], bf16, tag="xT")
                        nc.sync.dma_start(
                            out=xT_sb[:],
                            in_=xT[:, b * HB_N + g0:b * HB_N + g0 + 6272])
                        for h0 in range(0, 6272, 512):
                            n = min(512, 6272 - h0)
                            ps = pbppool.tile([128, 512], f32, tag="pbp")
                            for q0 in range(0, n, 128):
                                nc.tensor.matmul(
                                    ps[:, q0:q0 + 128], A_sb[:],
                                    xT_sb[:, h0 + q0:h0 + q0 + 128],
                                    start=True, stop=True)
                            nc.scalar.activation(
                                tbl[:, g0 + h0:g0 + h0 + n, 0],
                                ps[:, 0:n], AF.Copy)

                    GHB = CHUNKS_PER_HB * (GQ // 16)
                    gidx_sb = gxpool.tile([128, GHB], i16, tag="gidx")
                    nc.sync.dma_start(
                        out=gidx_sb[:],
                        in_=gidx_w[:, b * GHB:(b + 1) * GHB])

                    if os.environ.get("SKIP_P2"):
                        continue
                    for c in range(CHUNKS_PER_HB):
                        gt = gpool.tile([128, GQ, 1], f32, tag="gt")
                        nc.gpsimd.ap_gather(
                            gt[:], tbl[:],
                            gidx_sb[:, c * (GQ // 16):
                                    (c + 1) * (GQ // 16)],
                            128, HB_N, 1, GQ)

                        comb = eapool.tile([F + EAR, CHUNK], bf16,
                                           tag="comb")
                        for q in range(4):
                            a0, a1 = QBOUND[q]
                            if (c + q) % 2 == 0:
                                nc.vector.tensor_copy(
                                    out=comb[0:F, a0:a1],
                                    in_=gt[32 * q:32 * q + 32, 0:a1 - a0, 0])
                            else:
                                nc.scalar.activation(
                                    comb[0:F, a0:a1],
                                    gt[32 * q:32 * q + 32, 0:a1 - a0, 0],
                                    AF.Copy)
                        slot0 = b * HB_SLOTS + c * CHUNK
                        nc.sync.dma_start(out=comb[F:F + EAR, :],
                                          in_=eaT_w[:, slot0:slot0 + CHUNK])

                        for ci in range(CHUNK_CELLS):
                            dblk = c * CHUNK_CELLS + ci
                            zb = ebppool.tile([128, TPB * (HID + 1)], f32,
                                              tag="ebp")
                            for t in range(TPB):
                                s = ci * CELL + t * 128       # chunk slot
                                nc.tensor.matmul(
                                    zb[:, t * (HID + 1):(t + 1) * (HID + 1)],
                                    comb[:, s:s + 128],
                                    I64_sb[0:F + EAR, :],
                                    start=True, stop=True)
                            zv = zb[:].rearrange("p (t e) -> p t e", e=HID + 1)
                            msg = mpool.tile([128, TPB, HID + 1], bf16,
                                             tag="msg")
                            nc.scalar.activation(msg[:], zv, AF.Relu)

                            oh = ohpool.tile([128, TPB, 128], bf16, tag="oh")
                            tcol = (b * NBLK + dblk) * TPB
                            o_ap = offs_sb[:, tcol:tcol + TPB]
                            o_b = bass.AP(o_ap.tensor, o_ap.offset,
                                          [o_ap.ap[0], o_ap.ap[1], [0, 128]])
                            i_ap = iota_sb[:]
                            i_b = bass.AP(i_ap.tensor, i_ap.offset,
                                          [i_ap.ap[0], [0, TPB], i_ap.ap[1]])
                            nc.vector.tensor_tensor(out=oh[:], in0=o_b,
                                                    in1=i_b, op=OP.is_equal)

                            if ci == 0:
                                accp = accppool.tile(
                                    [HID + 1, CHUNK_CELLS * 128], f32,
                                    tag="accp")
                            for t in range(TPB):
                                nc.tensor.matmul(
                                    accp[:, ci * 128:(ci + 1) * 128],
                                    msg[:, t, :], oh[:, t, :],
                                    start=(t == 0),
                                    stop=(t == TPB - 1))
                            if ci == CHUNK_CELLS - 1:
                                d0 = c * CHUNK_CELLS * 128
                                d1 = d0 + CHUNK_CELLS * 128
                                nc.vector.tensor_tensor(
                                    out=acc_sb[0:HID + 1, d0:d1],
                                    in0=acc_sb[0:HID + 1, d0:d1],
                                    in1=accp[:], op=OP.add)

            # ---- Phase 3: node MLP (streamed, 256 nodes per group) ----
            with tc.tile_pool(name="node", bufs=3) as npool, \
                 tc.tile_pool(name="nodep", bufs=1, space="PSUM") as nppool:
                for G in range(NSH_PAD // 256):         # 49 groups
                    # rows: 0:64 Agg^T, 64 ind^T, 65:97 x_own^T
                    rhs2 = npool.tile([F + HID + 1, 256], f32, tag="rhs2")
                    nc.sync.dma_start(out=rhs2[HID + 1:HID + 1 + F, :],
                                      in_=x_ownT[:, G * 256:(G + 1) * 256])
                    for j in range(2):
                        q = 2 * G + j
                        pS = nppool.tile([128, 128], f32, tag="pS")
                        nc.tensor.transpose(
                            out=pS[:], in_=acc_sb[:, q * 128:(q + 1) * 128],
                            identity=idf[:])
                        rec = npool.tile([128, 1], f32, tag="rec")
                        nc.vector.tensor_scalar_max(rec[:], pS[:, HID:HID + 1],
                                                    1.0)
                        nc.vector.reciprocal(rec[:], rec[:])
                        ind = npool.tile([128, 1], f32, tag="ind")
                        nc.vector.tensor_scalar_min(ind[:], pS[:, HID:HID + 1],
                                                    1.0)
                        pA = nppool.tile([128, HID], f32, tag="pA")
                        nc.tensor.matmul(pA[:],
                                         acc_sb[0:HID, q * 128:(q + 1) * 128],
                                         W1b_sb[:], start=True, stop=True)
                        tq = npool.tile([128, 128], f32, tag="tq")
                        nc.vector.tensor_scalar_mul(tq[:, 0:HID], pA[:], rec[:])
                        nc.vector.tensor_copy(out=tq[:, HID:HID + 1], in_=ind[:])
                        nc.vector.memset(tq[:, HID + 1:128], 0.0)
                        pT = nppool.tile([128, 128], f32, tag="pT")
                        nc.tensor.transpose(out=pT[:], in_=tq[:], identity=idf[:])
                        nc.vector.tensor_copy(
                            out=rhs2[0:HID + 1, j * 128:(j + 1) * 128],
                            in_=pT[0:HID + 1, :])
                    ph = nppool.tile([HID, 256], f32, tag="ph")
                    nc.tensor.matmul(ph[:], W2a_sb[:], rhs2[:],
                                     start=True, stop=True)
                    h1 = npool.tile([HID, 256], bf16, tag="h1")
                    nc.scalar.activation(h1[:], ph[:], AF.Relu, bias=b2a_sb[:])
                    po = nppool.tile([NTGT, 256], f32, tag="po")
                    nc.tensor.matmul(po[:], W2b_sb[:], h1[:],
                                     start=True, stop=True)
                    ot = npool.tile([NTGT, 256], f32, tag="ot")
                    nc.scalar.activation(ot[:], po[:], AF.Identity,
                                         bias=b2b_sb[:])
                    nc.sync.dma_start(out=outT[:, G * 256:(G + 1) * 256],
                                      in_=ot[:])

    nc.compile()
    return nc


def _host_prep(x, edge_index, edge_attr, W1a, b1a, W1b, b1b, W2a, b2a,
               W2b, b2b):
    bf = ml_dtypes.bfloat16
    row = np.asarray(edge_index[0], dtype=np.int64)
    col = np.asarray(edge_index[1], dtype=np.int64)
    x = np.asarray(x, dtype=np.float32)
    ea = np.asarray(edge_attr, dtype=np.float32)

    xT_pad = np.zeros((F, HB * HB_N), bf)
    xT_pad[:, :N_NODES] = x.T

    # merged z moving operand: rows 0:32 = W1a_top, rows 32:65 = Btil
    # (W1a ea-part + b1a row with count-indicator col)
    I64e = np.zeros((128, HID + 1), np.float32)
    I64e[0:F, 0:HID] = W1a[:F, :]
    I64e[F:F + F, 0:HID] = W1a[F:, :]
    I64e[2 * F, 0:HID] = b1a
    I64e[2 * F, HID] = 1.0

    Btil = np.zeros((EAR, HID + 1), np.float32)
    Btil[0:F, 0:HID] = W1a[F:, :]
    Btil[F, 0:HID] = b1a
    Btil[F, HID] = 1.0

    # rhs2 row order: [Agg (64); ind (1); x (32)]
    W2a_eff = np.concatenate(
        [W2a[F:, :], (b1b @ W2a[F:, :]).reshape(1, HID), W2a[:F, :]],
        axis=0).astype(np.float32)

    iota = np.tile(np.arange(128, dtype=np.float32), (128, 1)).astype(bf)

    common = {
        "xT": xT_pad,
        "A_w": np.tile(np.eye(F, dtype=np.float32), (1, 4)).astype(bf),
        "I64_w": I64e.astype(bf),
        "Btil_w": Btil.astype(bf),
        "W1b_w": np.asarray(W1b, np.float32),
        "W2a_w": W2a_eff,
        "b2a_w": np.asarray(b2a, np.float32).reshape(HID, 1),
        "b2b_w": np.asarray(b2b, np.float32).reshape(NTGT, 1),
        "W2b_w": np.asarray(W2b, np.float32).astype(bf),
        "iota_w": iota,
        "idf_w": np.eye(128, dtype=np.float32),
    }

    shard = row // NSH
    in_maps = []
    for core in range(NCORES):
        sel = np.nonzero(shard == core)[0]
        lrow = (row[sel] - core * NSH).astype(np.int64)
        scol = col[sel]
        hb = scol // HB_N
        lcol = (scol - hb * HB_N).astype(np.int64)
        blk = lrow >> 7
        bid = hb * NBLK + blk
        order = np.lexsort((lrow, bid))
        sbid = bid[order]
        cnt = np.bincount(bid, minlength=NCELL_TOT)
        mx = cnt.max()
        assert mx <= CELL, f"cell overflow: {mx} > {CELL}"
        starts = np.zeros(NCELL_TOT + 1, np.int64)
        starts[1:] = np.cumsum(cnt)
        within = np.arange(len(order)) - starts[sbid]
        slots = sbid * CELL + within

        gidx = np.zeros(S_TOT, np.int16)          # poison -> node 0
        gidx[slots] = lcol[order].astype(np.int16)
        eaT = np.zeros((EAR, S_TOT), bf)
        eaT[0:F, slots] = ea[sel][order].T
        eaT[F, slots] = 1.0
        offs = np.full(S_TOT, OFFS_NONE, np.float32)
        offs[slots] = (lrow[order] & 127).astype(np.float32)

        # gather idx columns, 4-way core-group split: quarter q's Q7 pair
        # (partitions 32q:32q+32) serves chunk slots QBOUND[q], padded to GQ
        nchunks = S_TOT // CHUNK
        gw = np.zeros((128, nchunks * (GQ // 16)), np.int16)
        gcv = gidx.reshape(nchunks, CHUNK)
        for ck in range(nchunks):
            for q, (a0, a1) in enumerate(QBOUND):
                qi = np.zeros(GQ, np.int16)
                qi[:a1 - a0] = gcv[ck, a0:a1]
                w = qi.reshape(GQ // 16, 16).T        # [16, 48]
                gw[32 * q:32 * q + 32,
                   ck * (GQ // 16):(ck + 1) * (GQ // 16)] = np.tile(w, (2, 1))

        x_ownT = np.zeros((F, NSH_PAD), np.float32)
        x_ownT[:, :NSH] = x[core * NSH:(core + 1) * NSH].T

        m = dict(common)
        m["x_ownT"] = x_ownT
        m["gidx_w"] = gw
        m["eaT_w"] = eaT
        m["offs_w"] = offs.reshape(S_TOT // 128, 128).T.astype(bf).copy()
        in_maps.append(m)
    return in_maps


def kernel(x, edge_index, edge_attr, u, batch,
           W1a, b1a, W1b, b1b, W2a, b2a, W2b, b2b, _profile=False):
    from concourse import bass_utils

    if "nc" not in _CACHE:
        _CACHE["nc"] = _build_nc()
    nc = _CACHE["nc"]

    in_maps = _host_prep(x, edge_index, edge_attr, W1a, b1a, W1b, b1b,
                         W2a, b2a, W2b, b2b)
    import os as _os
    if _os.environ.get("BASS_SIM"):
        from concourse.bass_interp import CoreSim
        sim = CoreSim(nc, trace=False)
        for name, arr in in_maps[0].items():
            sim.tensor(name)[:] = arr
        sim.simulate()
        outT = np.asarray(sim.tensor("outT"))
        out = np.zeros((N_NODES, NTGT), np.float32)
        out[:NSH] = outT[:, :NSH].T
        return out
    res = bass_utils.run_bass_kernel_spmd(
        nc, in_maps, core_ids=list(range(NCORES)), trace=_profile)
    out = np.empty((N_NODES, NTGT), np.float32)
    for core in range(NCORES):
        out[core * NSH:(core + 1) * NSH] = \
            res.results[core]["outT"][:, :NSH].T
    if _profile:
        _CACHE["last_exec_time_ns"] = res.exec_time_ns
    return out



# revision 9
# speedup vs baseline: 3.3224x; 1.0374x over previous
"""GNN message-passing (NodeModel) Trainium2 kernel, 8 NeuronCores.

Sharding: edges partitioned by destination node (12500 nodes/core) -> the
segment-sum stays core-local, no collectives. Per core, edges are grouped by
source half-bank (8 x 12544 nodes), sorted by destination, and laid out in
fixed 384-slot cells per (half-bank, 128-dest-block) so every core runs the
identical program (SPMD, one NEFF).

Math (W1b deferred out of the edge loop):
  z_e   = P[col_e] + ea_e @ B + b1a,   P = x @ W1a[:32]
  m_e   = relu(z_e);  S_n = sum_e m_e;  c_n = deg(n)
  agg_n = (S_n / max(c_n,1)) @ W1b + 1[c_n>0] * b1b
  out   = relu([x | agg] @ W2a + b2a) @ W2b + b2b

The per-edge P fetch avoids DMA gather entirely (random 256B HBM gathers
measured ~31 GB/s, latency-bound): P^T for one half-bank lives in SBUF
feature-major [64, 12544] f32 (computed on device), and gpsimd ap_gather
expands it per-slot along the free dim. Partitions 64:128 hold a copy of the
table so the 8 Q7 cores serve two slot-halves per gathered column (column c
-> rows 0:64 = slot c, rows 64:128 = slot c + CHUNK/2). z is accumulated in
PSUM from two matmuls (gathered-P against I64, edge attrs against Btil); S
and c are accumulated with one-hot matmuls into PSUM; b1b is folded into
W2a_eff with the count indicator as a 97th feature.
"""
import numpy as np
import ml_dtypes

N_NODES = 100000
F = 32
HID = 64
NTGT = 32
NCORES = 8

NSH = 12500            # dest nodes per core
NSH_PAD = 12544        # 98 * 128
NBLK = 98              # dest-node blocks (128 nodes) per core
HB = 8                 # source half-banks
HB_N = 12544           # source nodes per half-bank
CELL = 384             # slots per (half-bank, dest-block) = 3 tiles of 128
TPB = 3
NCELL_TOT = HB * NBLK              # 784
S_TOT = NCELL_TOT * CELL           # 301056
CHUNK_CELLS = 7
CHUNK = CHUNK_CELLS * CELL         # 2688 slots (= one gather call)
CHUNKS_PER_HB = NBLK // CHUNK_CELLS    # 14
HB_SLOTS = NBLK * CELL                 # 37632
OFFS_NONE = 200.0
GQ = 768               # gathered columns per chunk (4-way core-group split)
# chunk tile -> (quarter, col offset): quarters cover (6,5,5,5) tiles
QMAP = {}
for _t in range(21):
    if _t < 6:
        QMAP[_t] = (0, _t * 128)
    elif _t < 11:
        QMAP[_t] = (1, (_t - 6) * 128)
    elif _t < 16:
        QMAP[_t] = (2, (_t - 11) * 128)
    else:
        QMAP[_t] = (3, (_t - 16) * 128)
QBOUND = [(0, 768), (768, 1408), (1408, 2048), (2048, 2688)]
EAR = F + 1            # ea rows (32 feats + indicator)

_CACHE = {}


def _build_nc():
    import os
    import concourse.bass as bass
    import concourse.bacc as bacc
    import concourse.mybir as mybir
    from concourse.tile import TileContext
    from bass_rust import add_dep_helper

    f32 = mybir.dt.float32
    bf16 = mybir.dt.bfloat16
    i16 = mybir.dt.int16
    AF = mybir.ActivationFunctionType
    OP = mybir.AluOpType

    nc = bacc.Bacc("TRN2", target_bir_lowering=False, debug=False,
                   num_devices=NCORES)

    xT = nc.dram_tensor("xT", [F, HB * HB_N], bf16, kind="ExternalInput")
    x_ownT = nc.dram_tensor("x_ownT", [F, NSH_PAD], f32, kind="ExternalInput")
    A_w = nc.dram_tensor("A_w", [F, 128], bf16, kind="ExternalInput")
    I64_w = nc.dram_tensor("I64_w", [128, HID + 1], bf16, kind="ExternalInput")
    Btil_w = nc.dram_tensor("Btil_w", [EAR, HID + 1], bf16,
                            kind="ExternalInput")
    W1b_w = nc.dram_tensor("W1b_w", [HID, HID], f32, kind="ExternalInput")
    W2a_w = nc.dram_tensor("W2a_w", [F + HID + 1, HID], f32, kind="ExternalInput")
    b2a_w = nc.dram_tensor("b2a_w", [HID, 1], f32, kind="ExternalInput")
    W2b_w = nc.dram_tensor("W2b_w", [HID, NTGT], bf16, kind="ExternalInput")
    b2b_w = nc.dram_tensor("b2b_w", [NTGT, 1], f32, kind="ExternalInput")
    iota_w = nc.dram_tensor("iota_w", [128, 128], bf16, kind="ExternalInput")
    idf_w = nc.dram_tensor("idf_w", [128, 128], f32, kind="ExternalInput")
    gidx_w = nc.dram_tensor("gidx_w", [128, S_TOT // CHUNK * (GQ // 16)], i16,
                            kind="ExternalInput")
    eaT_w = nc.dram_tensor("eaT_w", [EAR, S_TOT], bf16, kind="ExternalInput")
    offs_w = nc.dram_tensor("offs_w", [128, S_TOT // 128], bf16,
                            kind="ExternalInput")
    outT = nc.dram_tensor("outT", [NTGT, NSH_PAD], f32, kind="ExternalOutput")

    with TileContext(nc) as tc:
        with tc.tile_pool(name="const", bufs=1) as cpool, \
             tc.tile_pool(name="acc", bufs=1) as apool, \
             tc.tile_pool(name="tbl", bufs=2) as tpool:

            # constants
            A_sb = cpool.tile([F, 128], bf16)
            nc.sync.dma_start(out=A_sb[:], in_=A_w[:])
            I64_sb = cpool.tile([128, HID + 1], bf16)
            nc.sync.dma_start(out=I64_sb[:], in_=I64_w[:])
            Btil_sb = cpool.tile([EAR, HID + 1], bf16)
            nc.sync.dma_start(out=Btil_sb[:], in_=Btil_w[:])
            W1b_sb = cpool.tile([HID, HID], f32)
            nc.sync.dma_start(out=W1b_sb[:], in_=W1b_w[:])
            W2a_sb = cpool.tile([F + HID + 1, HID], f32)
            nc.sync.dma_start(out=W2a_sb[:], in_=W2a_w[:])
            b2a_sb = cpool.tile([HID, 1], f32)
            nc.sync.dma_start(out=b2a_sb[:], in_=b2a_w[:])
            W2b_sb = cpool.tile([HID, NTGT], bf16)
            nc.sync.dma_start(out=W2b_sb[:], in_=W2b_w[:])
            b2b_sb = cpool.tile([NTGT, 1], f32)
            nc.sync.dma_start(out=b2b_sb[:], in_=b2b_w[:])
            iota_sb = cpool.tile([128, 128], bf16)
            nc.sync.dma_start(out=iota_sb[:], in_=iota_w[:])
            offs_sb = cpool.tile([128, S_TOT // 128], bf16)
            nc.sync.dma_start(out=offs_sb[:], in_=offs_w[:])
            idf = cpool.tile([128, 128], f32)
            nc.sync.dma_start(out=idf[:], in_=idf_w[:])

            acc_sb = apool.tile([128, NSH_PAD], f32)
            nc.vector.memset(acc_sb[:], 0.0)

            with tc.tile_pool(name="xtp", bufs=1) as xpool, \
                 tc.tile_pool(name="pbp", bufs=1, space="PSUM") as pbppool, \
                 tc.tile_pool(name="gidx", bufs=2) as gxpool, \
                 tc.tile_pool(name="gt", bufs=2) as gpool, \
                 tc.tile_pool(name="ea", bufs=2) as eapool, \
                 tc.tile_pool(name="msg", bufs=4) as mpool, \
                 tc.tile_pool(name="oh", bufs=4) as ohpool, \
                 tc.tile_pool(name="ebp", bufs=3, space="PSUM") as ebppool, \
                 tc.tile_pool(name="accp", bufs=2, space="PSUM") as accppool:

                for b in range(HB):
                    tbl = tpool.tile([128, HB_N, 1], f32, tag="tbl")
                    # ---- table build: x^T replicated 4x, feature-major ----
                    for g0 in range(0, HB_N, 6272):
                        xT_sb = xpool.tile([F, 6272], bf16, tag="xT")
                        nc.sync.dma_start(
                            out=xT_sb[:],
                            in_=xT[:, b * HB_N + g0:b * HB_N + g0 + 6272])
                        for h0 in range(0, 6272, 512):
                            n = min(512, 6272 - h0)
                            ps = pbppool.tile([128, 512], f32, tag="pbp")
                            for q0 in range(0, n, 128):
                                nc.tensor.matmul(
                                    ps[:, q0:q0 + 128], A_sb[:],
                                    xT_sb[:, h0 + q0:h0 + q0 + 128],
                                    start=True, stop=True)
                            nc.scalar.activation(
                                tbl[:, g0 + h0:g0 + h0 + n, 0],
                                ps[:, 0:n], AF.Copy)

                    GHB = CHUNKS_PER_HB * (GQ // 16)
                    gidx_sb = gxpool.tile([128, GHB], i16, tag="gidx")
                    nc.sync.dma_start(
                        out=gidx_sb[:],
                        in_=gidx_w[:, b * GHB:(b + 1) * GHB])

                    if os.environ.get("SKIP_P2"):
                        continue
                    for cc in range(CHUNKS_PER_HB // 2):
                      gt = gpool.tile([128, 2 * GQ, 1], f32, tag="gt")
                      nc.gpsimd.ap_gather(
                          gt[:], tbl[:],
                          gidx_sb[:, cc * 2 * (GQ // 16):
                                  (cc + 1) * 2 * (GQ // 16)],
                          128, HB_N, 1, 2 * GQ)
                      for c in (2 * cc, 2 * cc + 1):
                        gof = (c - 2 * cc) * GQ
                        comb = eapool.tile([F + EAR, CHUNK], bf16,
                                           tag="comb")
                        for q in range(4):
                            a0, a1 = QBOUND[q]
                            if (c + q) % 2 == 0:
                                nc.vector.tensor_copy(
                                    out=comb[0:F, a0:a1],
                                    in_=gt[32 * q:32 * q + 32,
                                           gof:gof + a1 - a0, 0])
                            else:
                                nc.scalar.activation(
                                    comb[0:F, a0:a1],
                                    gt[32 * q:32 * q + 32,
                                       gof:gof + a1 - a0, 0],
                                    AF.Copy)
                        slot0 = b * HB_SLOTS + c * CHUNK
                        nc.sync.dma_start(out=comb[F:F + EAR, :],
                                          in_=eaT_w[:, slot0:slot0 + CHUNK])

                        for ci in range(CHUNK_CELLS):
                            dblk = c * CHUNK_CELLS + ci
                            zb = ebppool.tile([128, TPB * (HID + 1)], f32,
                                              tag="ebp")
                            for t in range(TPB):
                                s = ci * CELL + t * 128       # chunk slot
                                nc.tensor.matmul(
                                    zb[:, t * (HID + 1):(t + 1) * (HID + 1)],
                                    comb[:, s:s + 128],
                                    I64_sb[0:F + EAR, :],
                                    start=True, stop=True)
                            zv = zb[:].rearrange("p (t e) -> p t e", e=HID + 1)
                            msg = mpool.tile([128, TPB, HID + 1], bf16,
                                             tag="msg")
                            nc.scalar.activation(msg[:], zv, AF.Relu)

                            oh = ohpool.tile([128, TPB, 128], bf16, tag="oh")
                            tcol = (b * NBLK + dblk) * TPB
                            o_ap = offs_sb[:, tcol:tcol + TPB]
                            o_b = bass.AP(o_ap.tensor, o_ap.offset,
                                          [o_ap.ap[0], o_ap.ap[1], [0, 128]])
                            i_ap = iota_sb[:]
                            i_b = bass.AP(i_ap.tensor, i_ap.offset,
                                          [i_ap.ap[0], [0, TPB], i_ap.ap[1]])
                            nc.vector.tensor_tensor(out=oh[:], in0=o_b,
                                                    in1=i_b, op=OP.is_equal)

                            if ci == 0:
                                accp = accppool.tile(
                                    [HID + 1, CHUNK_CELLS * 128], f32,
                                    tag="accp")
                            for t in range(TPB):
                                nc.tensor.matmul(
                                    accp[:, ci * 128:(ci + 1) * 128],
                                    msg[:, t, :], oh[:, t, :],
                                    start=(t == 0),
                                    stop=(t == TPB - 1))
                            if ci == CHUNK_CELLS - 1:
                                d0 = c * CHUNK_CELLS * 128
                                d1 = d0 + CHUNK_CELLS * 128
                                nc.vector.tensor_tensor(
                                    out=acc_sb[0:HID + 1, d0:d1],
                                    in0=acc_sb[0:HID + 1, d0:d1],
                                    in1=accp[:], op=OP.add)

            # ---- Phase 3: node MLP (streamed, 256 nodes per group) ----
            with tc.tile_pool(name="node", bufs=3) as npool, \
                 tc.tile_pool(name="nodep", bufs=1, space="PSUM") as nppool:
                for G in range(NSH_PAD // 256):         # 49 groups
                    # rows: 0:64 Agg^T, 64 ind^T, 65:97 x_own^T
                    rhs2 = npool.tile([F + HID + 1, 256], f32, tag="rhs2")
                    nc.sync.dma_start(out=rhs2[HID + 1:HID + 1 + F, :],
                                      in_=x_ownT[:, G * 256:(G + 1) * 256])
                    for j in range(2):
                        q = 2 * G + j
                        pS = nppool.tile([128, 128], f32, tag="pS")
                        nc.tensor.transpose(
                            out=pS[:], in_=acc_sb[:, q * 128:(q + 1) * 128],
                            identity=idf[:])
                        rec = npool.tile([128, 1], f32, tag="rec")
                        nc.vector.tensor_scalar_max(rec[:], pS[:, HID:HID + 1],
                                                    1.0)
                        nc.vector.reciprocal(rec[:], rec[:])
                        ind = npool.tile([128, 1], f32, tag="ind")
                        nc.vector.tensor_scalar_min(ind[:], pS[:, HID:HID + 1],
                                                    1.0)
                        pA = nppool.tile([128, HID], f32, tag="pA")
                        nc.tensor.matmul(pA[:],
                                         acc_sb[0:HID, q * 128:(q + 1) * 128],
                                         W1b_sb[:], start=True, stop=True)
                        tq = npool.tile([128, 128], f32, tag="tq")
                        nc.vector.tensor_scalar_mul(tq[:, 0:HID], pA[:], rec[:])
                        nc.vector.tensor_copy(out=tq[:, HID:HID + 1], in_=ind[:])
                        nc.vector.memset(tq[:, HID + 1:128], 0.0)
                        pT = nppool.tile([128, 128], f32, tag="pT")
                        nc.tensor.transpose(out=pT[:], in_=tq[:], identity=idf[:])
                        nc.vector.tensor_copy(
                            out=rhs2[0:HID + 1, j * 128:(j + 1) * 128],
                            in_=pT[0:HID + 1, :])
                    ph = nppool.tile([HID, 256], f32, tag="ph")
                    nc.tensor.matmul(ph[:], W2a_sb[:], rhs2[:],
                                     start=True, stop=True)
                    h1 = npool.tile([HID, 256], bf16, tag="h1")
                    nc.scalar.activation(h1[:], ph[:], AF.Relu, bias=b2a_sb[:])
                    po = nppool.tile([NTGT, 256], f32, tag="po")
                    nc.tensor.matmul(po[:], W2b_sb[:], h1[:],
                                     start=True, stop=True)
                    ot = npool.tile([NTGT, 256], f32, tag="ot")
                    nc.scalar.activation(ot[:], po[:], AF.Identity,
                                         bias=b2b_sb[:])
                    nc.sync.dma_start(out=outT[:, G * 256:(G + 1) * 256],
                                      in_=ot[:])

    nc.compile()
    return nc


def _host_prep(x, edge_index, edge_attr, W1a, b1a, W1b, b1b, W2a, b2a,
               W2b, b2b):
    bf = ml_dtypes.bfloat16
    row = np.asarray(edge_index[0], dtype=np.int64)
    col = np.asarray(edge_index[1], dtype=np.int64)
    x = np.asarray(x, dtype=np.float32)
    ea = np.asarray(edge_attr, dtype=np.float32)

    xT_pad = np.zeros((F, HB * HB_N), bf)
    xT_pad[:, :N_NODES] = x.T

    # merged z moving operand: rows 0:32 = W1a_top, rows 32:65 = Btil
    # (W1a ea-part + b1a row with count-indicator col)
    I64e = np.zeros((128, HID + 1), np.float32)
    I64e[0:F, 0:HID] = W1a[:F, :]
    I64e[F:F + F, 0:HID] = W1a[F:, :]
    I64e[2 * F, 0:HID] = b1a
    I64e[2 * F, HID] = 1.0

    Btil = np.zeros((EAR, HID + 1), np.float32)
    Btil[0:F, 0:HID] = W1a[F:, :]
    Btil[F, 0:HID] = b1a
    Btil[F, HID] = 1.0

    # rhs2 row order: [Agg (64); ind (1); x (32)]
    W2a_eff = np.concatenate(
        [W2a[F:, :], (b1b @ W2a[F:, :]).reshape(1, HID), W2a[:F, :]],
        axis=0).astype(np.float32)

    iota = np.tile(np.arange(128, dtype=np.float32), (128, 1)).astype(bf)

    common = {
        "xT": xT_pad,
        "A_w": np.tile(np.eye(F, dtype=np.float32), (1, 4)).astype(bf),
        "I64_w": I64e.astype(bf),
        "Btil_w": Btil.astype(bf),
        "W1b_w": np.asarray(W1b, np.float32),
        "W2a_w": W2a_eff,
        "b2a_w": np.asarray(b2a, np.float32).reshape(HID, 1),
        "b2b_w": np.asarray(b2b, np.float32).reshape(NTGT, 1),
        "W2b_w": np.asarray(W2b, np.float32).astype(bf),
        "iota_w": iota,
        "idf_w": np.eye(128, dtype=np.float32),
    }

    shard = row // NSH
    in_maps = []
    for core in range(NCORES):
        sel = np.nonzero(shard == core)[0]
        lrow = (row[sel] - core * NSH).astype(np.int64)
        scol = col[sel]
        hb = scol // HB_N
        lcol = (scol - hb * HB_N).astype(np.int64)
        blk = lrow >> 7
        bid = hb * NBLK + blk
        order = np.lexsort((lrow, bid))
        sbid = bid[order]
        cnt = np.bincount(bid, minlength=NCELL_TOT)
        mx = cnt.max()
        assert mx <= CELL, f"cell overflow: {mx} > {CELL}"
        starts = np.zeros(NCELL_TOT + 1, np.int64)
        starts[1:] = np.cumsum(cnt)
        within = np.arange(len(order)) - starts[sbid]
        slots = sbid * CELL + within

        gidx = np.zeros(S_TOT, np.int16)          # poison -> node 0
        gidx[slots] = lcol[order].astype(np.int16)
        eaT = np.zeros((EAR, S_TOT), bf)
        eaT[0:F, slots] = ea[sel][order].T
        eaT[F, slots] = 1.0
        offs = np.full(S_TOT, OFFS_NONE, np.float32)
        offs[slots] = (lrow[order] & 127).astype(np.float32)

        # gather idx columns, 4-way core-group split: quarter q's Q7 pair
        # (partitions 32q:32q+32) serves chunk slots QBOUND[q], padded to GQ
        nchunks = S_TOT // CHUNK
        gw = np.zeros((128, nchunks * (GQ // 16)), np.int16)
        gcv = gidx.reshape(nchunks, CHUNK)
        for ck in range(nchunks):
            for q, (a0, a1) in enumerate(QBOUND):
                qi = np.zeros(GQ, np.int16)
                qi[:a1 - a0] = gcv[ck, a0:a1]
                w = qi.reshape(GQ // 16, 16).T        # [16, 48]
                gw[32 * q:32 * q + 32,
                   ck * (GQ // 16):(ck + 1) * (GQ // 16)] = np.tile(w, (2, 1))

        x_ownT = np.zeros((F, NSH_PAD), np.float32)
        x_ownT[:, :NSH] = x[core * NSH:(core + 1) * NSH].T

        m = dict(common)
        m["x_ownT"] = x_ownT
        m["gidx_w"] = gw
        m["eaT_w"] = eaT
        m["offs_w"] = offs.reshape(S_TOT // 128, 128).T.astype(bf).copy()
        in_maps.append(m)
    return in_maps


def kernel(x, edge_index, edge_attr, u, batch,
           W1a, b1a, W1b, b1b, W2a, b2a, W2b, b2b, _profile=False):
    from concourse import bass_utils

    if "nc" not in _CACHE:
        _CACHE["nc"] = _build_nc()
    nc = _CACHE["nc"]

    in_maps = _host_prep(x, edge_index, edge_attr, W1a, b1a, W1b, b1b,
                         W2a, b2a, W2b, b2b)
    import os as _os
    if _os.environ.get("BASS_SIM"):
        from concourse.bass_interp import CoreSim
        sim = CoreSim(nc, trace=False)
        for name, arr in in_maps[0].items():
            sim.tensor(name)[:] = arr
        sim.simulate()
        outT = np.asarray(sim.tensor("outT"))
        out = np.zeros((N_NODES, NTGT), np.float32)
        out[:NSH] = outT[:, :NSH].T
        return out
    res = bass_utils.run_bass_kernel_spmd(
        nc, in_maps, core_ids=list(range(NCORES)), trace=_profile)
    out = np.empty((N_NODES, NTGT), np.float32)
    for core in range(NCORES):
        out[core * NSH:(core + 1) * NSH] = \
            res.results[core]["outT"][:, :NSH].T
    if _profile:
        _CACHE["last_exec_time_ns"] = res.exec_time_ns
    return out



# revision 10
# speedup vs baseline: 3.6459x; 1.0974x over previous
"""GNN message-passing (NodeModel) Trainium2 kernel, 8 NeuronCores.

Sharding: edges partitioned by destination node (12500 nodes/core) -> the
segment-sum stays core-local, no collectives. Per core, edges are grouped by
source half-bank (8 x 12544 nodes), sorted by destination, and laid out in
fixed 384-slot cells per (half-bank, 128-dest-block) so every core runs the
identical program (SPMD, one NEFF).

Math (W1b deferred out of the edge loop):
  z_e   = P[col_e] + ea_e @ B + b1a,   P = x @ W1a[:32]
  m_e   = relu(z_e);  S_n = sum_e m_e;  c_n = deg(n)
  agg_n = (S_n / max(c_n,1)) @ W1b + 1[c_n>0] * b1b
  out   = relu([x | agg] @ W2a + b2a) @ W2b + b2b

The per-edge P fetch avoids DMA gather entirely (random 256B HBM gathers
measured ~31 GB/s, latency-bound): P^T for one half-bank lives in SBUF
feature-major [64, 12544] f32 (computed on device), and gpsimd ap_gather
expands it per-slot along the free dim. Partitions 64:128 hold a copy of the
table so the 8 Q7 cores serve two slot-halves per gathered column (column c
-> rows 0:64 = slot c, rows 64:128 = slot c + CHUNK/2). z is accumulated in
PSUM from two matmuls (gathered-P against I64, edge attrs against Btil); S
and c are accumulated with one-hot matmuls into PSUM; b1b is folded into
W2a_eff with the count indicator as a 97th feature.
"""
import numpy as np
import ml_dtypes

N_NODES = 100000
F = 32
HID = 64
NTGT = 32
NCORES = 8

NSH = 12500            # dest nodes per core
NSH_PAD = 12544        # 98 * 128
NBLK = 98              # dest-node blocks (128 nodes) per core
HB = 8                 # source half-banks
HB_N = 12544           # source nodes per half-bank
CELL = 384             # slots per (half-bank, dest-block) = 3 tiles of 128
TPB = 3
NCELL_TOT = HB * NBLK              # 784
S_TOT = NCELL_TOT * CELL           # 301056
CHUNK_CELLS = 7
CHUNK = CHUNK_CELLS * CELL         # 2688 slots (= one gather call)
CHUNKS_PER_HB = NBLK // CHUNK_CELLS    # 14
HB_SLOTS = NBLK * CELL                 # 37632
OFFS_NONE = 200.0
GQ = 672               # gathered columns per chunk (4-way core-group split)
# exact even quarters: the casts unpack into the flat comb column space,
# so quarter boundaries need no tile alignment
QBOUND = [(0, 672), (672, 1344), (1344, 2016), (2016, 2688)]
EAR = F + 1            # ea rows (32 feats + indicator)

_CACHE = {}


def _build_nc():
    import os
    import concourse.bass as bass
    import concourse.bacc as bacc
    import concourse.mybir as mybir
    from concourse.tile import TileContext
    from bass_rust import add_dep_helper

    f32 = mybir.dt.float32
    bf16 = mybir.dt.bfloat16
    i16 = mybir.dt.int16
    AF = mybir.ActivationFunctionType
    OP = mybir.AluOpType

    nc = bacc.Bacc("TRN2", target_bir_lowering=False, debug=False,
                   num_devices=NCORES)

    xT = nc.dram_tensor("xT", [F, HB * HB_N], bf16, kind="ExternalInput")
    x_ownT = nc.dram_tensor("x_ownT", [F, NSH_PAD], f32, kind="ExternalInput")
    A_w = nc.dram_tensor("A_w", [F, 128], bf16, kind="ExternalInput")
    I64_w = nc.dram_tensor("I64_w", [128, HID + 1], bf16, kind="ExternalInput")
    Btil_w = nc.dram_tensor("Btil_w", [EAR, HID + 1], bf16,
                            kind="ExternalInput")
    W1b_w = nc.dram_tensor("W1b_w", [HID, HID], f32, kind="ExternalInput")
    W2a_w = nc.dram_tensor("W2a_w", [F + HID + 1, HID], f32, kind="ExternalInput")
    b2a_w = nc.dram_tensor("b2a_w", [HID, 1], f32, kind="ExternalInput")
    W2b_w = nc.dram_tensor("W2b_w", [HID, NTGT], bf16, kind="ExternalInput")
    b2b_w = nc.dram_tensor("b2b_w", [NTGT, 1], f32, kind="ExternalInput")
    iota_w = nc.dram_tensor("iota_w", [128, 128], bf16, kind="ExternalInput")
    idf_w = nc.dram_tensor("idf_w", [128, 128], f32, kind="ExternalInput")
    gidx_w = nc.dram_tensor("gidx_w", [128, S_TOT // CHUNK * (GQ // 16)], i16,
                            kind="ExternalInput")
    eaT_w = nc.dram_tensor("eaT_w", [EAR, S_TOT], bf16, kind="ExternalInput")
    offs_w = nc.dram_tensor("offs_w", [128, S_TOT // 128], bf16,
                            kind="ExternalInput")
    outT = nc.dram_tensor("outT", [NTGT, NSH_PAD], f32, kind="ExternalOutput")

    with TileContext(nc) as tc:
        with tc.tile_pool(name="const", bufs=1) as cpool, \
             tc.tile_pool(name="acc", bufs=1) as apool, \
             tc.tile_pool(name="tbl", bufs=2) as tpool:

            # constants
            A_sb = cpool.tile([F, 128], bf16)
            nc.sync.dma_start(out=A_sb[:], in_=A_w[:])
            I64_sb = cpool.tile([128, HID + 1], bf16)
            nc.sync.dma_start(out=I64_sb[:], in_=I64_w[:])
            Btil_sb = cpool.tile([EAR, HID + 1], bf16)
            nc.sync.dma_start(out=Btil_sb[:], in_=Btil_w[:])
            W1b_sb = cpool.tile([HID, HID], f32)
            nc.sync.dma_start(out=W1b_sb[:], in_=W1b_w[:])
            W2a_sb = cpool.tile([F + HID + 1, HID], f32)
            nc.sync.dma_start(out=W2a_sb[:], in_=W2a_w[:])
            b2a_sb = cpool.tile([HID, 1], f32)
            nc.sync.dma_start(out=b2a_sb[:], in_=b2a_w[:])
            W2b_sb = cpool.tile([HID, NTGT], bf16)
            nc.sync.dma_start(out=W2b_sb[:], in_=W2b_w[:])
            b2b_sb = cpool.tile([NTGT, 1], f32)
            nc.sync.dma_start(out=b2b_sb[:], in_=b2b_w[:])
            iota_sb = cpool.tile([128, 128], bf16)
            nc.sync.dma_start(out=iota_sb[:], in_=iota_w[:])
            offs_sb = cpool.tile([128, S_TOT // 128], bf16)
            nc.sync.dma_start(out=offs_sb[:], in_=offs_w[:])
            idf = cpool.tile([128, 128], f32)
            nc.sync.dma_start(out=idf[:], in_=idf_w[:])

            acc_sb = apool.tile([128, NSH_PAD], f32)
            nc.vector.memset(acc_sb[:], 0.0)

            with tc.tile_pool(name="xtp", bufs=1) as xpool, \
                 tc.tile_pool(name="pbp", bufs=1, space="PSUM") as pbppool, \
                 tc.tile_pool(name="gidx", bufs=2) as gxpool, \
                 tc.tile_pool(name="gt", bufs=2) as gpool, \
                 tc.tile_pool(name="ea", bufs=2) as eapool, \
                 tc.tile_pool(name="msg", bufs=4) as mpool, \
                 tc.tile_pool(name="oh", bufs=4) as ohpool, \
                 tc.tile_pool(name="ebp", bufs=3, space="PSUM") as ebppool, \
                 tc.tile_pool(name="accp", bufs=2, space="PSUM") as accppool:

                for b in range(HB):
                    tbl = tpool.tile([128, HB_N, 1], f32, tag="tbl")
                    # ---- table build: x^T replicated 4x, feature-major ----
                    for g0 in range(0, HB_N, 6272):
                        xT_sb = xpool.tile([F, 6272], bf16, tag="xT")
                        nc.sync.dma_start(
                            out=xT_sb[:],
                            in_=xT[:, b * HB_N + g0:b * HB_N + g0 + 6272])
                        for h0 in range(0, 6272, 512):
                            n = min(512, 6272 - h0)
                            ps = pbppool.tile([128, 512], f32, tag="pbp")
                            for q0 in range(0, n, 128):
                                nc.tensor.matmul(
                                    ps[:, q0:q0 + 128], A_sb[:],
                                    xT_sb[:, h0 + q0:h0 + q0 + 128],
                                    start=True, stop=True)
                            nc.scalar.activation(
                                tbl[:, g0 + h0:g0 + h0 + n, 0],
                                ps[:, 0:n], AF.Copy)

                    GHB = CHUNKS_PER_HB * (GQ // 16)
                    gidx_sb = gxpool.tile([128, GHB], i16, tag="gidx")
                    nc.sync.dma_start(
                        out=gidx_sb[:],
                        in_=gidx_w[:, b * GHB:(b + 1) * GHB])

                    if os.environ.get("SKIP_P2"):
                        continue
                    for cc in range(CHUNKS_PER_HB // 2):
                      gt = gpool.tile([128, 2 * GQ, 1], f32, tag="gt")
                      nc.gpsimd.ap_gather(
                          gt[:], tbl[:],
                          gidx_sb[:, cc * 2 * (GQ // 16):
                                  (cc + 1) * 2 * (GQ // 16)],
                          128, HB_N, 1, 2 * GQ)
                      for c in (2 * cc, 2 * cc + 1):
                        gof = (c - 2 * cc) * GQ
                        comb = eapool.tile([F + EAR, CHUNK], bf16,
                                           tag="comb")
                        for q in range(4):
                            a0, a1 = QBOUND[q]
                            if (c + q) % 2 == 0:
                                nc.vector.tensor_copy(
                                    out=comb[0:F, a0:a1],
                                    in_=gt[32 * q:32 * q + 32,
                                           gof:gof + a1 - a0, 0])
                            else:
                                nc.scalar.activation(
                                    comb[0:F, a0:a1],
                                    gt[32 * q:32 * q + 32,
                                       gof:gof + a1 - a0, 0],
                                    AF.Copy)
                        slot0 = b * HB_SLOTS + c * CHUNK
                        nc.sync.dma_start(out=comb[F:F + EAR, :],
                                          in_=eaT_w[:, slot0:slot0 + CHUNK])

                        for ci in range(CHUNK_CELLS):
                            dblk = c * CHUNK_CELLS + ci
                            zb = ebppool.tile([128, TPB * (HID + 1)], f32,
                                              tag="ebp")
                            for t in range(TPB):
                                s = ci * CELL + t * 128       # chunk slot
                                nc.tensor.matmul(
                                    zb[:, t * (HID + 1):(t + 1) * (HID + 1)],
                                    comb[:, s:s + 128],
                                    I64_sb[0:F + EAR, :],
                                    start=True, stop=True)
                            zv = zb[:].rearrange("p (t e) -> p t e", e=HID + 1)
                            msg = mpool.tile([128, TPB, HID + 1], bf16,
                                             tag="msg")
                            nc.scalar.activation(msg[:], zv, AF.Relu)

                            oh = ohpool.tile([128, TPB, 128], bf16, tag="oh")
                            tcol = (b * NBLK + dblk) * TPB
                            o_ap = offs_sb[:, tcol:tcol + TPB]
                            o_b = bass.AP(o_ap.tensor, o_ap.offset,
                                          [o_ap.ap[0], o_ap.ap[1], [0, 128]])
                            i_ap = iota_sb[:]
                            i_b = bass.AP(i_ap.tensor, i_ap.offset,
                                          [i_ap.ap[0], [0, TPB], i_ap.ap[1]])
                            nc.vector.tensor_tensor(out=oh[:], in0=o_b,
                                                    in1=i_b, op=OP.is_equal)

                            if ci == 0:
                                accp = accppool.tile(
                                    [HID + 1, CHUNK_CELLS * 128], f32,
                                    tag="accp")
                            for t in range(TPB):
                                nc.tensor.matmul(
                                    accp[:, ci * 128:(ci + 1) * 128],
                                    msg[:, t, :], oh[:, t, :],
                                    start=(t == 0),
                                    stop=(t == TPB - 1))
                            if ci == CHUNK_CELLS - 1:
                                d0 = c * CHUNK_CELLS * 128
                                d1 = d0 + CHUNK_CELLS * 128
                                nc.vector.tensor_tensor(
                                    out=acc_sb[0:HID + 1, d0:d1],
                                    in0=acc_sb[0:HID + 1, d0:d1],
                                    in1=accp[:], op=OP.add)

            # ---- Phase 3: node MLP (streamed, 256 nodes per group) ----
            with tc.tile_pool(name="node", bufs=3) as npool, \
                 tc.tile_pool(name="nodep", bufs=1, space="PSUM") as nppool:
                for G in range(NSH_PAD // 256):         # 49 groups
                    # rows: 0:64 Agg^T, 64 ind^T, 65:97 x_own^T
                    rhs2 = npool.tile([F + HID + 1, 256], f32, tag="rhs2")
                    nc.sync.dma_start(out=rhs2[HID + 1:HID + 1 + F, :],
                                      in_=x_ownT[:, G * 256:(G + 1) * 256])
                    for j in range(2):
                        q = 2 * G + j
                        pS = nppool.tile([128, 128], f32, tag="pS")
                        nc.tensor.transpose(
                            out=pS[:], in_=acc_sb[:, q * 128:(q + 1) * 128],
                            identity=idf[:])
                        rec = npool.tile([128, 1], f32, tag="rec")
                        nc.vector.tensor_scalar_max(rec[:], pS[:, HID:HID + 1],
                                                    1.0)
                        nc.vector.reciprocal(rec[:], rec[:])
                        ind = npool.tile([128, 1], f32, tag="ind")
                        nc.vector.tensor_scalar_min(ind[:], pS[:, HID:HID + 1],
                                                    1.0)
                        pA = nppool.tile([128, HID], f32, tag="pA")
                        nc.tensor.matmul(pA[:],
                                         acc_sb[0:HID, q * 128:(q + 1) * 128],
                                         W1b_sb[:], start=True, stop=True)
                        tq = npool.tile([128, 128], f32, tag="tq")
                        nc.vector.tensor_scalar_mul(tq[:, 0:HID], pA[:], rec[:])
                        nc.vector.tensor_copy(out=tq[:, HID:HID + 1], in_=ind[:])
                        nc.vector.memset(tq[:, HID + 1:128], 0.0)
                        pT = nppool.tile([128, 128], f32, tag="pT")
                        nc.tensor.transpose(out=pT[:], in_=tq[:], identity=idf[:])
                        nc.vector.tensor_copy(
                            out=rhs2[0:HID + 1, j * 128:(j + 1) * 128],
                            in_=pT[0:HID + 1, :])
                    ph = nppool.tile([HID, 256], f32, tag="ph")
                    nc.tensor.matmul(ph[:], W2a_sb[:], rhs2[:],
                                     start=True, stop=True)
                    h1 = npool.tile([HID, 256], bf16, tag="h1")
                    nc.scalar.activation(h1[:], ph[:], AF.Relu, bias=b2a_sb[:])
                    po = nppool.tile([NTGT, 256], f32, tag="po")
                    nc.tensor.matmul(po[:], W2b_sb[:], h1[:],
                                     start=True, stop=True)
                    ot = npool.tile([NTGT, 256], f32, tag="ot")
                    nc.scalar.activation(ot[:], po[:], AF.Identity,
                                         bias=b2b_sb[:])
                    nc.sync.dma_start(out=outT[:, G * 256:(G + 1) * 256],
                                      in_=ot[:])

    nc.compile()
    return nc


def _host_prep(x, edge_index, edge_attr, W1a, b1a, W1b, b1b, W2a, b2a,
               W2b, b2b):
    bf = ml_dtypes.bfloat16
    row = np.asarray(edge_index[0], dtype=np.int64)
    col = np.asarray(edge_index[1], dtype=np.int64)
    x = np.asarray(x, dtype=np.float32)
    ea = np.asarray(edge_attr, dtype=np.float32)

    xT_pad = np.zeros((F, HB * HB_N), bf)
    xT_pad[:, :N_NODES] = x.T

    # merged z moving operand: rows 0:32 = W1a_top, rows 32:65 = Btil
    # (W1a ea-part + b1a row with count-indicator col)
    I64e = np.zeros((128, HID + 1), np.float32)
    I64e[0:F, 0:HID] = W1a[:F, :]
    I64e[F:F + F, 0:HID] = W1a[F:, :]
    I64e[2 * F, 0:HID] = b1a
    I64e[2 * F, HID] = 1.0

    Btil = np.zeros((EAR, HID + 1), np.float32)
    Btil[0:F, 0:HID] = W1a[F:, :]
    Btil[F, 0:HID] = b1a
    Btil[F, HID] = 1.0

    # rhs2 row order: [Agg (64); ind (1); x (32)]
    W2a_eff = np.concatenate(
        [W2a[F:, :], (b1b @ W2a[F:, :]).reshape(1, HID), W2a[:F, :]],
        axis=0).astype(np.float32)

    iota = np.tile(np.arange(128, dtype=np.float32), (128, 1)).astype(bf)

    common = {
        "xT": xT_pad,
        "A_w": np.tile(np.eye(F, dtype=np.float32), (1, 4)).astype(bf),
        "I64_w": I64e.astype(bf),
        "Btil_w": Btil.astype(bf),
        "W1b_w": np.asarray(W1b, np.float32),
        "W2a_w": W2a_eff,
        "b2a_w": np.asarray(b2a, np.float32).reshape(HID, 1),
        "b2b_w": np.asarray(b2b, np.float32).reshape(NTGT, 1),
        "W2b_w": np.asarray(W2b, np.float32).astype(bf),
        "iota_w": iota,
        "idf_w": np.eye(128, dtype=np.float32),
    }

    shard = row // NSH
    in_maps = []
    for core in range(NCORES):
        sel = np.nonzero(shard == core)[0]
        lrow = (row[sel] - core * NSH).astype(np.int64)
        scol = col[sel]
        hb = scol // HB_N
        lcol = (scol - hb * HB_N).astype(np.int64)
        blk = lrow >> 7
        bid = hb * NBLK + blk
        order = np.lexsort((lrow, bid))
        sbid = bid[order]
        cnt = np.bincount(bid, minlength=NCELL_TOT)
        mx = cnt.max()
        assert mx <= CELL, f"cell overflow: {mx} > {CELL}"
        starts = np.zeros(NCELL_TOT + 1, np.int64)
        starts[1:] = np.cumsum(cnt)
        within = np.arange(len(order)) - starts[sbid]
        slots = sbid * CELL + within

        gidx = np.zeros(S_TOT, np.int16)          # poison -> node 0
        gidx[slots] = lcol[order].astype(np.int16)
        eaT = np.zeros((EAR, S_TOT), bf)
        eaT[0:F, slots] = ea[sel][order].T
        eaT[F, slots] = 1.0
        offs = np.full(S_TOT, OFFS_NONE, np.float32)
        offs[slots] = (lrow[order] & 127).astype(np.float32)

        # gather idx columns, 4-way core-group split: quarter q's Q7 pair
        # (partitions 32q:32q+32) serves chunk slots QBOUND[q], padded to GQ
        nchunks = S_TOT // CHUNK
        gw = np.zeros((128, nchunks * (GQ // 16)), np.int16)
        gcv = gidx.reshape(nchunks, CHUNK)
        for ck in range(nchunks):
            for q, (a0, a1) in enumerate(QBOUND):
                qi = np.zeros(GQ, np.int16)
                qi[:a1 - a0] = gcv[ck, a0:a1]
                w = qi.reshape(GQ // 16, 16).T        # [16, 48]
                gw[32 * q:32 * q + 32,
                   ck * (GQ // 16):(ck + 1) * (GQ // 16)] = np.tile(w, (2, 1))

        x_ownT = np.zeros((F, NSH_PAD), np.float32)
        x_ownT[:, :NSH] = x[core * NSH:(core + 1) * NSH].T

        m = dict(common)
        m["x_ownT"] = x_ownT
        m["gidx_w"] = gw
        m["eaT_w"] = eaT
        m["offs_w"] = offs.reshape(S_TOT // 128, 128).T.astype(bf).copy()
        in_maps.append(m)
    return in_maps


def kernel(x, edge_index, edge_attr, u, batch,
           W1a, b1a, W1b, b1b, W2a, b2a, W2b, b2b, _profile=False):
    from concourse import bass_utils

    if "nc" not in _CACHE:
        _CACHE["nc"] = _build_nc()
    nc = _CACHE["nc"]

    in_maps = _host_prep(x, edge_index, edge_attr, W1a, b1a, W1b, b1b,
                         W2a, b2a, W2b, b2b)
    import os as _os
    if _os.environ.get("BASS_SIM"):
        from concourse.bass_interp import CoreSim
        sim = CoreSim(nc, trace=False)
        for name, arr in in_maps[0].items():
            sim.tensor(name)[:] = arr
        sim.simulate()
        outT = np.asarray(sim.tensor("outT"))
        out = np.zeros((N_NODES, NTGT), np.float32)
        out[:NSH] = outT[:, :NSH].T
        return out
    res = bass_utils.run_bass_kernel_spmd(
        nc, in_maps, core_ids=list(range(NCORES)), trace=_profile)
    out = np.empty((N_NODES, NTGT), np.float32)
    for core in range(NCORES):
        out[core * NSH:(core + 1) * NSH] = \
            res.results[core]["outT"][:, :NSH].T
    if _profile:
        _CACHE["last_exec_time_ns"] = res.exec_time_ns
    return out

